# revision 1
# baseline (speedup 1.0000x reference)
"""Trainium2 Bass kernel for AttentionAggregator (GNN message passing).

Reference computation:
    new_emb = fb @ W + b
    s_e     = (fa @ a1)[src_e] + (new_emb @ a2)[dst_e]
    score_e = exp(elu(s_e, 0.1))
    out[n]  = (sum_{e: src_e=n} score_e * new_emb[dst_e]) / max(den[n], den==0->1)

Algebraic reformulation used here (linearity of the segment sum):
    q_e   = fb[dst_e] @ (W @ a2)            # per-edge scalar
    s_e   = (fa @ a1)[src_e] + q_e + b @ a2
    G[n]  = sum_e score_e * fb[dst_e]       # [Na, 64]
    den[n]= sum_e score_e
    out[n]= (G[n] / den_safe[n]) @ W + 1[den[n] > 0] * b

so new_emb is never materialized; only raw fb rows are gathered.

Distribution: nodes (and their incoming edge lists, after a host-side sort of
edges by src) are sharded contiguously across the 8 cores.  Each core owns
6272 output rows, gathers fb rows for its own edges (fb replicated), and no
cross-core collective is needed.

Device-side layout: each node's edge list is split into "virtual nodes" of at
most D0=24 slots.  A group of 128 virtual nodes occupies the 128 partitions;
their slot indices are gathered with one multi-index indirect DMA
([128, B*24] indices -> [128, B*24*64] fb rows for B groups per call).  Slot
scores are computed with per-slot ops, and the slot dimension is reduced with
strided-AP tensor_reduce.  A second tiny pass combines the <=KV virtual rows
of each real node, divides by den, and applies the final @W (+b) with the PE.
"""

import sys

for _p in ("/opt/trn_rl_repo",):
    if _p not in sys.path:
        sys.path.insert(0, _p)

import numpy as np

import concourse.bass as bass
import concourse.bacc as bacc
import concourse.mybir as mybir
import concourse.tile as tile
from concourse.bass import IndirectOffsetOnAxis
from concourse.masks import make_identity

P = 128
F = 64          # feature dim
D0 = 12         # edge slots per virtual node
NCORES = 8

f32 = mybir.dt.float32
bf16 = mybir.dt.bfloat16
i32 = mybir.dt.int32
i16 = mybir.dt.int16
TC = 16          # nodes per partition in the table-build pass
TW = 128         # augmented table row (bf16): fb (64) | q (1) | pad -> 256B
AX = mybir.AxisListType
OP = mybir.AluOpType
ACTF = mybir.ActivationFunctionType


# ----------------------------------------------------------------------------
# device program
# ----------------------------------------------------------------------------

def emit_program(tc, ins, outs, cfg):
    """Emit the per-core program.

    ins:  dict of APs: fb_tab [NB, F], pk [Gv, P, PKW], cpk [Gc, P, 2*KV],
          wvec [P, 3*F], wmat [F, F]
    outs: dict of APs: out [Gc*P, F], vtab [Gvp*P, VW] (scratch, Internal)
    cfg:  dict(Gv, Gc, KV, B, ba2)
    """
    nc = tc.nc
    Gv, Gc, KV, B = cfg["Gv"], cfg["Gc"], cfg["KV"], cfg["B"]
    ba2 = float(cfg["ba2"])
    fb_tab = ins["fb_tab"]
    pk = ins["pk"]
    cpk = ins["cpk"]
    wvec = ins["wvec"]
    wmat = ins["wmat"]
    out = outs["out"]
    vtab = outs["vtab"]
    tab2 = outs["tab2"]
    NB_pad = fb_tab.shape[0]

    gidx = ins["gidx"]
    cidx = ins["cidx"]
    PKW = F + D0              # fa_v row | slot mask
    NIa, H = cfg["NIa"], cfg["H"]
    VW = 128                  # vtab row (f32): G (64) | den (1) | pad -> 512B
    S = B * D0                # slots per phase-1 iteration
    NI = Gv // B
    assert Gv % B == 0

    with (
        tc.tile_pool(name="const", bufs=1) as cpool,
        tc.tile_pool(name="work", bufs=4) as pool,
        tc.tile_pool(name="psum", bufs=3, space="PSUM") as psum,
    ):
        wvec_t = cpool.tile([P, 3 * F], f32)
        nc.sync.dma_start(out=wvec_t[:], in_=wvec)
        wmat_t = cpool.tile([P, F], f32)
        nc.sync.dma_start(out=wmat_t[0:F, :], in_=wmat)
        nc.sync.dma_start(out=wmat_t[F:2 * F, :], in_=wmat)
        ident = cpool.tile([P, P], f32)
        make_identity(nc, ident[:])
        zbias = cpool.tile([P, 1], f32)
        nc.vector.memset(zbias[:], 0.0)
        mbias = cpool.tile([P, 1], f32)
        nc.vector.memset(mbias[:], -0.1)

        a1v = wvec_t[:, 0:F]
        w2v = wvec_t[:, F:2 * F]
        bv = wvec_t[:, 2 * F:3 * F]
        w2b = cpool.tile([P, F], bf16)
        nc.vector.tensor_copy(out=w2b[:], in_=w2v)

        # ---------------- phase 0: build augmented bf16 table [fb | q] -----
        abl = cfg.get("ablate", set())
        NTI = NB_pad // (P * TC) if "p0" not in abl else 0
        # tiles 0..NTA-1 cover table half A (rows [0, H)); phase-1 A-batches
        # only read half A, so half-B tiles can build concurrently with them.
        NTA = min(NTI, -(-H // (P * TC)))
        fb4 = fb_tab.rearrange("(j p c) f -> j p c f", p=P, c=TC)
        t24 = tab2.rearrange("(j p c) w -> j p c w", p=P, c=TC)

        def build_tile(j):
            fbb = pool.tile([P, TC * F], bf16, tag="fbb")
            fbb3 = fbb[:].rearrange("p (c f) -> p c f", f=F)
            nc.gpsimd.dma_start(out=fbb3, in_=fb4[j])  # SWDGE cast f32->bf16
            prodt = pool.tile([P, TC * F], bf16, tag="prodt")
            prodt3 = prodt[:].rearrange("p (c f) -> p c f", f=F)
            nc.vector.tensor_tensor(
                out=prodt3, in0=fbb3,
                in1=w2b[:, None, :].to_broadcast([P, TC, F]), op=OP.mult,
            )
            qt = pool.tile([P, TC], f32, tag="qt")
            nc.vector.tensor_reduce(
                out=qt[:], in_=prodt3, axis=AX.X, op=OP.add,
            )
            pck = pool.tile([P, TC * TW], bf16, tag="pck")
            pck3 = pck[:].rearrange("p (c w) -> p c w", w=TW)
            nc.vector.tensor_copy(out=pck3[:, :, F:F + 1], in_=qt[:, :, None])
            nc.vector.tensor_copy(out=pck3[:, :, 0:F], in_=fbb3)
            nc.sync.dma_start(out=t24[j][:, :, 0:F + 1], in_=pck3[:, :, 0:F + 1])

        for j in range(NTI):
            build_tile(j)
        tc.strict_bb_all_engine_barrier()

        # ---------------- phase 1: per-virtual-node-group segment sums -----
        def phase1_iter(it):
            g0 = it * B
            pk_t = pool.tile([P, B * PKW], f32, tag="pk")
            nc.sync.dma_start(
                out=pk_t[:].rearrange("p (b w) -> p b w", w=PKW),
                in_=pk[g0:g0 + B].rearrange("b p w -> p b w"),
            )
            pk3 = pk_t[:].rearrange("p (b w) -> p b w", w=PKW)
            gi_t = pool.tile([P, S * P // 16], i16, tag="gi", bufs=4)
            nc.sync.dma_start(out=gi_t[:], in_=gidx[it])

            rows = pool.tile([P, S * TW], bf16, tag="rows", bufs=4)
            rows3 = rows[:].rearrange("p (s w) -> p s w", w=TW)  # [P, S, 128]
            half = tab2[0:H, :] if it < NIa else tab2[H:2 * H, :]
            NIDX = cfg.get("nidx", 1024)      # per-call ring-capacity limit
            off = 0
            while off < S * P and "gather" not in abl:
                n = min(NIDX, S * P - off)
                nc.gpsimd.dma_gather(
                    out_ap=rows3[:, off // P:(off + n) // P, :],
                    in_ap=half,
                    idxs_ap=gi_t[:, off // 16:(off + n) // 16],
                    num_idxs=n,
                    num_idxs_reg=n,
                    elem_size=TW,
                )
                off += n
            # e1[p, b] = fa_v[p, b, :] @ a1
            fprod = pool.tile([P, B * F], f32, tag="fprod")
            nc.vector.tensor_tensor(
                out=fprod[:].rearrange("p (b f) -> p b f", f=F),
                in0=pk3[:, :, 0:F],
                in1=a1v[:, None, :].to_broadcast([P, B, F]),
                op=OP.mult,
            )
            e1 = pool.tile([P, B], f32, tag="e1")
            nc.vector.tensor_reduce(
                out=e1[:],
                in_=fprod[:].rearrange("p (b f) -> p b f", f=F),
                axis=AX.X, op=OP.add,
            )
            if ba2 != 0.0:
                nc.vector.tensor_scalar(
                    out=e1[:], in0=e1[:], scalar1=ba2, scalar2=None, op0=OP.add,
                )

            # s = q + e1; q is the gathered bf16 column 64 (+ ba2 in e1)
            s_t = pool.tile([P, S], f32, tag="s")
            nc.vector.tensor_tensor(
                out=s_t[:].rearrange("p (b k) -> p b k", k=D0),
                in0=rows3[:, :, F].rearrange("p (b k) -> p b k", k=D0),
                in1=e1[:, :, None].to_broadcast([P, B, D0]),
                op=OP.add,
            )

            # score = where(s + ba2 > 0, exp(s + ba2), exp(0.1*exp(s+ba2) - 0.1))
            t_t = pool.tile([P, S], f32, tag="t")
            nc.scalar.activation(t_t[:], s_t[:], ACTF.Exp, bias=zbias[:, 0:1],
                                 scale=1.0)
            u_t = pool.tile([P, S], f32, tag="u")
            nc.scalar.activation(u_t[:], t_t[:], ACTF.Exp, bias=mbias[:, 0:1],
                                 scale=0.1)
            m_t = pool.tile([P, S], mybir.dt.uint8, tag="m")
            nc.vector.tensor_scalar(
                out=m_t[:], in0=s_t[:], scalar1=0.0, scalar2=None, op0=OP.is_gt,
            )
            nc.vector.copy_predicated(out=u_t[:], mask=m_t[:], data=t_t[:])
            # zero padded slots and downcast to bf16 in one op
            u2 = pool.tile([P, S], bf16, tag="u2")
            nc.vector.tensor_tensor(
                out=u2[:].rearrange("p (b k) -> p b k", k=D0),
                in0=u_t[:].rearrange("p (b k) -> p b k", k=D0),
                in1=pk3[:, :, F:F + D0],
                op=OP.mult,
            )
            scaled = pool.tile([P, S * F], bf16, tag="scaled", bufs=2)
            scaled3 = scaled[:].rearrange("p (s f) -> p s f", f=F)
            vout = pool.tile([P, B * VW], f32, tag="vout")
            vout3 = vout[:].rearrange("p (b w) -> p b w", w=VW)
            if "big" not in abl:
                nc.vector.tensor_tensor(
                    out=scaled3,
                    in0=rows3[:, :, 0:F],
                    in1=u2[:, :, None].to_broadcast([P, S, F]),
                    op=OP.mult,
                )
                nc.vector.tensor_reduce(
                    out=vout3[:, :, 0:F],
                    in_=scaled[:].rearrange("p (b k f) -> p b f k", k=D0, f=F),
                    axis=AX.X, op=OP.add,
                )
            nc.vector.tensor_reduce(
                out=vout3[:, :, F:F + 1],
                in_=u2[:].rearrange("p (b k) -> p b k", k=D0),
                axis=AX.X, op=OP.add,
            )
            nc.sync.dma_start(
                out=vtab.rearrange("(g p) w -> g p w", p=P)[g0:g0 + B]
                    .rearrange("b p w -> p b w")[:, :, 0:F + 1],
                in_=vout3[:, :, 0:F + 1],
            )

        for it in range(NI):
            phase1_iter(it)

        # ---------------- phase 2: combine virtual rows, divide, @W + b ----
        B2 = cfg["B2"]
        Gc2 = cfg["Gc2"]
        out3 = out.rearrange("(g p) f -> g p f", p=P)
        for r2 in range(Gc2 // B2 if cfg.get("phases", "all") == "all" else 0):
            r0 = r2 * B2
            cpk_t = pool.tile([P, B2 * KV], f32, tag="cpk")
            nc.sync.dma_start(
                out=cpk_t[:].rearrange("p (b k) -> p b k", k=KV),
                in_=cpk[r0:r0 + B2].rearrange("b p k -> p b k"),
            )
            cm = cpk_t[:, 0:B2 * KV]
            ci_t = pool.tile([P, B2 * KV * P // 16], i16, tag="ci")
            nc.sync.dma_start(out=ci_t[:], in_=cidx[r2])

            gr = pool.tile([P, B2 * KV * VW], f32, tag="gr")
            gr3 = gr[:].rearrange("p (k w) -> p k w", w=VW)   # [P, B2*KV, VW]
            nc.gpsimd.dma_gather(
                out_ap=gr3,
                in_ap=vtab,
                idxs_ap=ci_t[:],
                num_idxs=B2 * KV * P,
                num_idxs_reg=B2 * KV * P,
                elem_size=VW,
            )

            scm = pool.tile([P, B2 * KV * (F + 1)], f32, tag="scm")
            nc.vector.tensor_tensor(
                out=scm[:].rearrange("p (k w) -> p k w", w=F + 1),
                in0=gr3[:, :, 0:F + 1],
                in1=cm[:, :, None].to_broadcast([P, B2 * KV, F + 1]),
                op=OP.mult,
            )
            hd = pool.tile([P, B2 * (F + 1)], f32, tag="hd")
            hd3 = hd[:].rearrange("p (b w) -> p b w", w=F + 1)
            nc.vector.tensor_reduce(
                out=hd3,
                in_=scm[:].rearrange("p (b k w) -> p b w k", k=KV, w=F + 1),
                axis=AX.X, op=OP.add,
            )
            den = hd3[:, :, F]                                 # [P, B2]
            m0 = pool.tile([P, B2], f32, tag="m0")
            nc.vector.tensor_scalar(
                out=m0[:], in0=den, scalar1=0.0, scalar2=None, op0=OP.is_equal,
            )
            dsafe = pool.tile([P, B2], f32, tag="dsafe")
            nc.vector.tensor_tensor(out=dsafe[:], in0=den, in1=m0[:], op=OP.add)
            rec = pool.tile([P, B2], f32, tag="rec")
            nc.vector.reciprocal(rec[:], dsafe[:])
            h_t = pool.tile([P, B2 * F], f32, tag="h")
            nc.vector.tensor_tensor(
                out=h_t[:].rearrange("p (b f) -> p b f", f=F),
                in0=hd3[:, :, 0:F],
                in1=rec[:, :, None].to_broadcast([P, B2, F]),
                op=OP.mult,
            )
            w1 = pool.tile([P, B2], f32, tag="w1")
            nc.vector.tensor_scalar(
                out=w1[:], in0=den, scalar1=0.0, scalar2=None, op0=OP.is_gt,
            )
            outs_t = pool.tile([P, B2 * F], f32, tag="outs")
            for b0 in range(0, B2, 2):
                bw = min(2, B2 - b0)
                htp = psum.tile([bw * F, P], f32, tag="htp")
                nc.tensor.transpose(
                    out=htp[:], in_=h_t[:, b0 * F:(b0 + bw) * F],
                    identity=ident[:])
                ht = pool.tile([bw * F, P], f32, tag="ht")
                nc.vector.tensor_copy(out=ht[:], in_=htp[:])
                for bb in range(bw):
                    b = b0 + bb
                    op_t = psum.tile([P, F], f32, tag="op")
                    nc.tensor.matmul(out=op_t[:],
                                     lhsT=ht[bb * F:(bb + 1) * F, :],
                                     rhs=wmat_t[bb * F:(bb + 1) * F, :],
                                     start=True, stop=True)
                    badd = pool.tile([P, F], f32, tag="badd")
                    nc.vector.tensor_scalar(
                        out=badd[:], in0=bv, scalar1=w1[:, b:b + 1],
                        scalar2=None, op0=OP.mult,
                    )
                    nc.vector.tensor_tensor(
                        out=outs_t[:, b * F:(b + 1) * F], in0=op_t[:],
                        in1=badd[:], op=OP.add,
                    )
            nc.sync.dma_start(
                out=out3[r0:r0 + B2].rearrange("g p f -> p g f"),
                in_=outs_t[:].rearrange("p (b f) -> p b f", f=F),
            )


# ----------------------------------------------------------------------------
# host-side preparation
# ----------------------------------------------------------------------------

def prep_inputs(feature_a, feature_b, W, b, a_vec, edges, node_num_a,
                ncores=NCORES, d0=D0):
    """Shard + pad inputs for the SPMD program.  Index plumbing only (sort,
    bincount, padding); the only host arithmetic is the tiny parameter
    derivation Wa2 = W @ a2 (64x64 matvec) and ba2 = b @ a2."""
    fa = np.asarray(feature_a, np.float32)
    fb = np.asarray(feature_b, np.float32)
    W = np.asarray(W, np.float32)
    b = np.asarray(b, np.float32)
    a_vec = np.asarray(a_vec, np.float32).reshape(-1)
    edges = np.asarray(edges)
    NA = int(node_num_a)
    NB, Fdim = fb.shape
    assert Fdim == F and fa.shape[1] == F

    src = edges[:, 0].astype(np.int64)
    dst = edges[:, 1].astype(np.int64)

    NB_pad = -(-NB // (P * TC)) * (P * TC)
    fb_pad = np.zeros((NB_pad, F), np.float32)
    fb_pad[:NB] = fb

    a1 = a_vec[:F]
    a2 = a_vec[F:]
    Wa2 = (W @ a2).astype(np.float32)
    ba2 = float(b @ a2)

    H = NB_pad // 2
    hflag = (dst >= H).astype(np.int64)
    order = np.lexsort((hflag, src))
    ssrc = src[order]
    sdst = dst[order].astype(np.int64)
    shf = hflag[order]
    deg = np.bincount(ssrc, minlength=NA).astype(np.int64)
    degA = np.bincount(ssrc[shf == 0], minlength=NA).astype(np.int64)
    degB = deg - degA
    row_ptr = np.zeros(NA + 1, np.int64)
    np.cumsum(deg, out=row_ptr[1:])

    nodes_per_core = -(-NA // (ncores * P)) * P          # 6272
    Gc = nodes_per_core // P                             # 49
    nvA = -(-degA // d0)
    nvB = -(-degB // d0)
    KV = max(2, int((nvA + nvB).max()))
    B = 4

    def build_half(lo, hi, degH, nvH, edge_off):
        """Virtual nodes for one dst-half of one core's node range.
        edge_off[n] = first sorted-edge position of this half's run."""
        n_nodes = max(hi - lo, 0)
        node_ids = np.arange(lo, hi)
        nvc = nvH[lo:hi] if n_nodes else np.zeros(0, np.int64)
        Nv = int(nvc.sum())
        vnode = np.repeat(node_ids, nvc)
        vstart0 = np.concatenate([[0], np.cumsum(nvc)])[:-1]
        vrank = np.arange(Nv) - np.repeat(vstart0, nvc)
        pos = edge_off[vnode][:, None] + vrank[:, None] * d0 + np.arange(d0)[None, :]
        valid = (vrank[:, None] * d0 + np.arange(d0)[None, :]) < degH[vnode][:, None]
        posc = np.clip(pos, 0, max(len(sdst) - 1, 0))
        sidx = np.where(valid, sdst[posc] if len(sdst) else 0, 0).astype(np.int64)
        return dict(Nv=Nv, vnode=vnode, nvc=nvc, vstart0=vstart0,
                    sidx=sidx, valid=valid)

    offA = row_ptr[:-1]            # A-run starts at the node's run start
    offB = row_ptr[:-1] + degA     # B-run follows
    cores = []
    for c in range(ncores):
        lo = c * nodes_per_core
        hi = min(lo + nodes_per_core, NA)
        ha = build_half(lo, hi, degA, nvA, offA)
        hb = build_half(lo, hi, degB, nvB, offB)
        hb["sidx"] = np.where(hb["valid"], hb["sidx"] - H, 0)
        cores.append((ha, hb))

    def cdiv(a, b):
        return -(-a // b)

    maxA = max(1, max(h[0]["Nv"] for h in cores))
    maxB = max(h[1]["Nv"] for h in cores)
    GvA = cdiv(cdiv(maxA, P), B) * B
    GvB = cdiv(cdiv(maxB, P), B) * B if maxB > 0 else 0
    NIa = GvA // B
    Gv = GvA + GvB
    Nvp = Gv * P
    B2 = min(4, max(1, 1024 // (KV * P)))
    Gc2 = cdiv(Gc, B2) * B2

    in_maps = []
    PKW = F + d0
    S = B * d0
    NI = Gv // B
    for c in range(ncores):
        ha, hb = cores[c]
        pk = np.zeros((Nvp, PKW), np.float32)
        sidx_all = np.zeros((Nvp, d0), np.int64)
        for (h, base) in ((ha, 0), (hb, GvA * P)):
            Nv = h["Nv"]
            if Nv:
                pk[base:base + Nv, 0:F] = fa[h["vnode"]]
                pk[base:base + Nv, F:F + d0] = h["valid"].astype(np.float32)
                sidx_all[base:base + Nv] = h["sidx"]
        pk = pk.reshape(Gv, P, PKW)

        # int16 gather indices: per batch, flat[(b*d0+k)*128 + p] =
        # sidx[group g0+b, partition p, slot k]; sbuf wrap [16, S*128//16],
        # replicated to 128 partitions; stored bitcast-f32 inside pk so one
        # DMA loads fa_v + mask + idx.  Device reads pk3[:, :, F+D0:] as the
        # per-iteration [P, S*P//16] i16 block, so per-group cols must hold
        # that group's quarter of the batch block: columns [b*d0*8*(..)].
        sidx_g = sidx_all.reshape(Gv, P, d0)
        gidx16 = np.zeros((NI, P, S * P // 16), np.int16)
        for i in range(NI):
            blk = sidx_g[i * B:(i + 1) * B]              # [B, P, d0]
            flat = blk.transpose(0, 2, 1).reshape(-1)    # [(b k) p]
            sb = flat.reshape(S * P // 16, 16).T.astype(np.int16)
            gidx16[i] = np.tile(sb, (8, 1))
        assert sidx_all.max() < 32768

        cpka = np.zeros((Gc2 * P, KV), np.float32)
        cidxa = np.zeros((Gc2 * P, KV), np.int64)
        n_nodes = min(nodes_per_core, NA - c * nodes_per_core)
        if n_nodes > 0:
            nv_tot = ha["nvc"] + hb["nvc"]
            ks = np.arange(KV)[None, :]
            cvalid = ks < nv_tot[:, None]
            # first the node's A-virtual rows, then its B-virtual rows
            inA = ks < ha["nvc"][:, None]
            idxA = ha["vstart0"][:, None] + ks
            idxB = GvA * P + hb["vstart0"][:, None] + (ks - ha["nvc"][:, None])
            cidxv = np.where(cvalid, np.where(inA, idxA, idxB), 0)
            cpka[:n_nodes, 0:KV] = cvalid.astype(np.float32)
            cidxa[:n_nodes] = cidxv
        cpk = cpka.reshape(Gc2, P, KV)
        assert cidxa.max() < 32768
        # int16 wrap for phase-2 dma_gather, one batch of B2 groups per call:
        # flat[(b*KV + k)*128 + p] = cidx[group r0+b, p, k]
        cg = cidxa.reshape(Gc2, P, KV)
        NW = B2 * KV * P // 16
        cidx16 = np.zeros((Gc2 // B2, P, NW), np.int16)
        for r in range(Gc2 // B2):
            flat = cg[r * B2:(r + 1) * B2].transpose(0, 2, 1).reshape(-1)
            sb = flat.reshape(NW, 16).T.astype(np.int16)
            cidx16[r] = np.tile(sb, (8, 1))

        wvec = np.zeros((P, 3 * F), np.float32)
        wvec[:, 0:F] = a1[None, :]
        wvec[:, F:2 * F] = Wa2[None, :]
        wvec[:, 2 * F:3 * F] = b[None, :]

        in_maps.append(dict(
            fb_tab=fb_pad,
            pk=np.ascontiguousarray(pk),
            gidx=np.ascontiguousarray(gidx16),
            cpk=np.ascontiguousarray(cpk),
            cidx=np.ascontiguousarray(cidx16),
            wvec=wvec,
            wmat=np.ascontiguousarray(W),
        ))

    cfg = dict(Gv=Gv, Gc=Gc, Gc2=Gc2, B2=B2, KV=KV, B=B, ba2=ba2, NB=NB,
               NB_pad=NB_pad, NIa=NIa, H=H, Nvp=Nvp,
               nodes_per_core=nodes_per_core, NA=NA)
    return in_maps, cfg


def build_bass(cfg, ncores=NCORES):
    nc = bacc.Bacc("TRN2", target_bir_lowering=False, debug=False,
                   enable_asserts=False, num_devices=ncores)
    ins = dict(
        fb_tab=nc.dram_tensor("fb_tab", [cfg["NB_pad"], F], f32,
                              kind="ExternalInput").ap(),
        pk=nc.dram_tensor("pk", [cfg["Gv"], P, F + D0], f32,
                          kind="ExternalInput").ap(),
        gidx=nc.dram_tensor("gidx", [cfg["Gv"] // cfg["B"], P,
                                     cfg["B"] * D0 * P // 16], i16,
                            kind="ExternalInput").ap(),
        cpk=nc.dram_tensor("cpk", [cfg["Gc2"], P, cfg["KV"]], f32,
                           kind="ExternalInput").ap(),
        cidx=nc.dram_tensor("cidx", [cfg["Gc2"] // cfg["B2"], P,
                                     cfg["B2"] * cfg["KV"] * P // 16], i16,
                            kind="ExternalInput").ap(),
        wvec=nc.dram_tensor("wvec", [P, 3 * F], f32, kind="ExternalInput").ap(),
        wmat=nc.dram_tensor("wmat", [F, F], f32, kind="ExternalInput").ap(),
    )
    outs = dict(
        out=nc.dram_tensor("out", [cfg["Gc2"] * P, F], f32,
                           kind="ExternalOutput").ap(),
        vtab=nc.dram_tensor("vtab", [cfg["Nvp"], 128], f32,
                            kind="ExternalOutput").ap(),
        tab2=nc.dram_tensor("tab2", [cfg["NB_pad"], TW], bf16,
                            kind="ExternalOutput").ap(),
    )
    with tile.TileContext(nc) as tc:
        emit_program(tc, ins, outs, cfg)
    nc.compile()
    return nc


# ----------------------------------------------------------------------------
# entry point
# ----------------------------------------------------------------------------

def kernel_with_results(trace=False, **inputs):
    from concourse import bass_utils

    in_maps, cfg = prep_inputs(**inputs)
    nc = build_bass(cfg)
    res = bass_utils.run_bass_kernel_spmd(
        nc, in_maps, core_ids=list(range(NCORES)), trace=trace,
    )
    outs = [r["out"][:cfg["nodes_per_core"]] for r in res.results]
    full = np.concatenate(outs, axis=0)[:cfg["NA"]]
    return full.astype(np.float32), res


def kernel(**inputs):
    return kernel_with_results(trace=False, **inputs)[0]


if __name__ == "__main__":
    np.random.seed(0)
    NA = NB = 50000
    E = 800000
    ins = dict(
        feature_a=np.random.randn(NA, F).astype(np.float32),
        feature_b=np.random.randn(NB, F).astype(np.float32),
        W=(np.random.randn(F, F) / 8).astype(np.float32),
        b=np.zeros(F, np.float32),
        a_vec=(np.random.randn(2 * F, 1) * 0.05).astype(np.float32),
        edges=np.stack([np.random.randint(0, NA, E),
                        np.random.randint(0, NB, E)], 1).astype(np.int64),
        node_num_a=NA,
    )
    out = kernel(**ins)
    print(out.shape, out.dtype)



# revision 2
# speedup vs baseline: 4001.3857x; 4001.3857x over previous
"""Trainium2 Bass kernel for AttentionAggregator (GNN message passing), v2.

Reference computation:
    new_emb = fb @ W + b
    s_e     = (fa @ a1)[src_e] + (new_emb @ a2)[dst_e]
    score_e = exp(elu(s_e, 0.1))
    out[n]  = (sum_{e: src_e=n} score_e * new_emb[dst_e]) / max(den[n], 1 if 0)

Reformulation (linearity of the segment sum):
    q_e   = fb[dst_e] @ (W @ a2)
    s_e   = (fa @ a1 + b @ a2)[src_e] + q_e
    G[n]  = sum_e score_e * fb[dst_e];  den[n] = sum_e score_e
    out[n]= (G[n] / den_safe[n]) @ W + 1[den>0] * b

Distribution: src nodes sharded contiguously across 8 cores (6272 rows
each); fb replicated.  No cross-core collective needed.

Device algorithm (per core):
  - Edges sorted by (dst-half, src).  Per src-group g (128 nodes) and
    dst-half X, edges are padded into chunks of 128 slots.  Chunk counts
    per (g, X) are cross-core uniform (max over cores) so one SPMD
    program serves all cores.
  - Chunks stream in batches of 8 (=1024 slots): one dma_gather per
    batch fetches fb[dst] f32 rows (256B) straight from the fb input,
    rotating over 4 SWDGE queues (~1.4 ns/row).
  - Per batch (vector/scalar engines): q = <row, W@a2>; one-hot
    onehot[e,n] = (srcloc[e]==n) via iota compare; e1 per edge via
    onehot x replicated-e1 reduce; score = exp(elu(q+e1, 0.1)); rhs65 =
    [score*fb | score] in bf16.
  - Segment sum on the PE: per chunk, matmul(onehot^T @ rhs65)
    accumulates [128 nodes x 65] (G | den) in a PSUM bank; group-major
    consumption keeps only ~3 accumulator banks live.
  - Epilogue per group: divide by den_safe, transpose via PE, multiply
    by W, add (den>0)*b, DMA out.
"""

import sys

for _p in ("/opt/trn_rl_repo",):
    if _p not in sys.path:
        sys.path.insert(0, _p)

import numpy as np

import concourse.bass as bass
import concourse.bacc as bacc
import concourse.mybir as mybir
import concourse.tile as tile
from concourse.masks import make_identity

P = 128
F = 64
NCORES = 8
CB = 8            # chunks per gather batch (8*128 = 1024 idxs)
NQ = 4            # SWDGE queues

f32 = mybir.dt.float32
bf16 = mybir.dt.bfloat16
i16 = mybir.dt.int16
u8 = mybir.dt.uint8
AX = mybir.AxisListType
OP = mybir.AluOpType
ACTF = mybir.ActivationFunctionType


# ----------------------------------------------------------------------------
# device program
# ----------------------------------------------------------------------------

def emit_program(tc, ins, outs, cfg):
    nc = tc.nc
    abl = cfg.get("ablate", set())
    Gc = cfg["Gc"]                      # 49 src groups
    H = cfg["H"]                        # rows per dst half
    meta = cfg["meta"]                  # per-half batch/chunk metadata
    ba2 = float(cfg["ba2"])
    fb_pad = ins["fb_pad"]              # [2H, F] f32
    fa_pk = ins["fa_pk"]                # [P, Gc*F] f32 (fa[g*128+p, f])
    gidx = {0: ins["gidxA"], 1: ins["gidxB"]}    # [NB_X, P, CB*P//16] i16
    gidx2 = {0: ins["gidx2A"], 1: ins["gidx2B"]}  # same layout, src-local idx
    sloc = {0: ins["slocA"], 1: ins["slocB"]}    # [NB_X, P, CB] f32
    iota8 = ins["iota8"]                # [P, CB*P] f32 (each part: 8x iota128)
    wvec = ins["wvec"]                  # [P, 3F] f32: a1 | Wa2 | b rows
    wmat = ins["wmat"]                  # [F, F] f32
    out = outs["out"]                   # [Gc*P, F] f32

    halves = {0: fb_pad[0:H, :], 1: fb_pad[H:2 * H, :]}

    with (
        tc.tile_pool(name="const", bufs=1) as cpool,
        tc.tile_pool(name="work", bufs=3) as pool,
        tc.tile_pool(name="psum", bufs=3, space="PSUM") as psum,
        tc.tile_pool(name="psep", bufs=2, space="PSUM") as psep,
    ):
        # ---------------- constants -----------------------------------
        wvec_t = cpool.tile([P, 3 * F], f32)
        nc.sync.dma_start(out=wvec_t[:], in_=wvec)
        a1v = wvec_t[:, 0:F]
        w2v = wvec_t[:, F:2 * F]
        bv = wvec_t[:, 2 * F:3 * F]
        wmat_t = cpool.tile([P, F], f32)
        nc.sync.dma_start(out=wmat_t[0:F, :], in_=wmat)
        iota_t = cpool.tile([P, CB * P], f32)
        nc.sync.dma_start(out=iota_t[:], in_=iota8)
        ident = cpool.tile([P, P], f32)
        make_identity(nc, ident[:])
        zbias = cpool.tile([P, 1], f32)
        nc.vector.memset(zbias[:], 0.0)
        mbias = cpool.tile([P, 1], f32)
        nc.vector.memset(mbias[:], -0.1)

        # ---------------- e1 = fa @ a1 + ba2, replicated --------------
        fa_t = cpool.tile([P, Gc * F], f32)
        nc.sync.dma_start(out=fa_t[:], in_=fa_pk)
        fprod = cpool.tile([P, Gc * F], f32)
        nc.vector.tensor_tensor(
            out=fprod[:].rearrange("p (g f) -> p g f", f=F),
            in0=fa_t[:].rearrange("p (g f) -> p g f", f=F),
            in1=a1v[:, None, :].to_broadcast([P, Gc, F]), op=OP.mult)
        e1_all = cpool.tile([P, Gc], f32)
        nc.vector.tensor_reduce(
            out=e1_all[:],
            in_=fprod[:].rearrange("p (g f) -> p g f", f=F),
            axis=AX.X, op=OP.add)
        nc.vector.tensor_scalar(
            out=e1_all[:], in0=e1_all[:], scalar1=ba2, scalar2=None,
            op0=OP.add)
        # e1tab[g*128+p, 0:F] = e1[g*128+p] (row-replicated) in DRAM, so
        # per-edge e1 comes from a second dma_gather stream (idx = src-local)
        e1sb = cpool.tile([P, Gc * F], f32)
        nc.vector.tensor_scalar(
            out=e1sb[:].rearrange("p (g f) -> p g f", f=F),
            in0=e1_all[:, :, None].to_broadcast([P, Gc, F]),
            scalar1=0.0, scalar2=None, op0=OP.add)
        e1tab = ins["e1tab"]
        nc.sync.dma_start(
            out=e1tab.rearrange("(g p) f -> p g f", p=P),
            in_=e1sb[:].rearrange("p (g f) -> p g f", f=F))
        tc.strict_bb_all_engine_barrier()

        # ---------------- per-batch compute ---------------------------
        batch_tiles = {}
        gather_ctr = [0]

        def get_batch(X, b):
            key = (X, b)
            if key in batch_tiles:
                return batch_tiles[key]
            gi = pool.tile([P, CB * P // 16], i16, tag=f"gi{X}")
            nc.sync.dma_start(out=gi[:], in_=gidx[X][b])
            gi2 = pool.tile([P, CB * P // 16], i16, tag=f"gi2{X}")
            nc.sync.dma_start(out=gi2[:], in_=gidx2[X][b])
            sl = pool.tile([P, CB], f32, tag=f"sl{X}")
            nc.sync.dma_start(out=sl[:], in_=sloc[X][b])
            rows = pool.tile([P, CB * F], f32, tag=f"rows{X}", bufs=4)
            rows3 = rows[:].rearrange("p (c f) -> p c f", f=F)
            if "nogather" not in abl:
                nc.gpsimd.dma_gather(
                    out_ap=rows3, in_ap=halves[X], idxs_ap=gi[:],
                    num_idxs=CB * P, num_idxs_reg=CB * P, elem_size=F,
                    queue_num=gather_ctr[0] % NQ)
            gather_ctr[0] += 1

            # q[p, c] = <rows[p, c, :], Wa2>
            qprod = pool.tile([P, CB * F], f32, tag=f"qp{X}")
            nc.vector.tensor_tensor(
                out=qprod[:].rearrange("p (c f) -> p c f", f=F),
                in0=rows3,
                in1=w2v[:, None, :].to_broadcast([P, CB, F]), op=OP.mult)
            q_t = pool.tile([P, CB], f32, tag=f"q{X}")
            nc.vector.tensor_reduce(
                out=q_t[:],
                in_=qprod[:].rearrange("p (c f) -> p c f", f=F),
                axis=AX.X, op=OP.add)

            # onehot[e, c, n] = (sloc[e, c] == n)
            oh = pool.tile([P, CB * P], bf16, tag=f"oh{X}")
            oh3 = oh[:].rearrange("p (c n) -> p c n", n=P)
            nc.vector.tensor_tensor(
                out=oh3,
                in0=sl[:, :, None].to_broadcast([P, CB, P]),
                in1=iota_t[:].rearrange("p (c n) -> p c n", n=P),
                op=OP.is_equal)

            # e1 per edge via gather from e1tab (idx = src-local node)
            e1g = pool.tile([P, CB * F], f32, tag=f"e1g{X}", bufs=4)
            e1g3 = e1g[:].rearrange("p (c f) -> p c f", f=F)
            if "noe1" not in abl:
                nc.gpsimd.dma_gather(
                    out_ap=e1g3, in_ap=e1tab, idxs_ap=gi2[:],
                    num_idxs=CB * P, num_idxs_reg=CB * P, elem_size=F,
                    queue_num=gather_ctr[0] % NQ)
                gather_ctr[0] += 1

            # score = exp(elu(q + e1, 0.1))
            s_t = pool.tile([P, CB], f32, tag=f"s{X}")
            nc.vector.tensor_tensor(out=s_t[:], in0=q_t[:],
                                    in1=e1g3[:, :, 0], op=OP.add)
            t_t = pool.tile([P, CB], f32, tag=f"t{X}")
            nc.scalar.activation(t_t[:], s_t[:], ACTF.Exp,
                                 bias=zbias[:, 0:1], scale=1.0)
            u_t = pool.tile([P, CB], f32, tag=f"u{X}")
            nc.scalar.activation(u_t[:], t_t[:], ACTF.Exp,
                                 bias=mbias[:, 0:1], scale=0.1)
            m_t = pool.tile([P, CB], u8, tag=f"m{X}")
            nc.vector.tensor_scalar(
                out=m_t[:], in0=s_t[:], scalar1=0.0, scalar2=None,
                op0=OP.is_gt)
            nc.vector.copy_predicated(out=u_t[:], mask=m_t[:], data=t_t[:])

            # rhs65[e, c, :] = [score * fb_row | score]  (bf16)
            rhs = pool.tile([P, CB * (F + 1)], bf16, tag=f"rhs{X}")
            rhs3 = rhs[:].rearrange("p (c w) -> p c w", w=F + 1)
            nc.vector.tensor_tensor(
                out=rhs3[:, :, 0:F], in0=rows3,
                in1=u_t[:, :, None].to_broadcast([P, CB, F]), op=OP.mult)
            nc.vector.tensor_copy(out=rhs3[:, :, F], in_=u_t[:])

            res = (oh3, rhs3)
            batch_tiles[key] = res
            # keep the dict small: drop entries older than a few batches
            for k in list(batch_tiles):
                if k[0] == X and k[1] < b - 2:
                    del batch_tiles[k]
            return res

        # ---------------- group-major matmul + epilogue ---------------
        out3 = out.rearrange("(g p) f -> g p f", p=P)
        outs_sb = None
        for g in range(Gc):
            acc = psum.tile([P, F + 1], f32, tag="acc")
            first = True
            for X in (0, 1):
                for (b, c) in meta[X]["group_chunks"][g]:
                    oh3, rhs3 = get_batch(X, b)
                    last = (X == 1) and (b, c) == meta[1]["group_chunks"][g][-1]
                    if "nomm" not in abl:
                        nc.tensor.matmul(
                            out=acc[:], lhsT=oh3[:, c, :], rhs=rhs3[:, c, :],
                            start=first, stop=last)
                    first = False

            # epilogue for group g
            if "noep" in abl or "nomm" in abl:
                continue
            den = acc[:, F:F + 1]
            m0 = pool.tile([P, 1], f32, tag="m0")
            nc.vector.tensor_scalar(out=m0[:], in0=den, scalar1=0.0,
                                    scalar2=None, op0=OP.is_equal)
            w1 = pool.tile([P, 1], f32, tag="w1")
            nc.vector.tensor_scalar(out=w1[:], in0=den, scalar1=0.0,
                                    scalar2=None, op0=OP.is_gt)
            dsafe = pool.tile([P, 1], f32, tag="dsafe")
            nc.vector.tensor_tensor(out=dsafe[:], in0=den, in1=m0[:],
                                    op=OP.add)
            rec = pool.tile([P, 1], f32, tag="rec")
            nc.vector.reciprocal(rec[:], dsafe[:])
            h_t = pool.tile([P, F], f32, tag="h")
            nc.vector.tensor_scalar(out=h_t[:], in0=acc[:, 0:F],
                                    scalar1=rec[:, 0:1], scalar2=None,
                                    op0=OP.mult)
            htp = psep.tile([F, P], f32, tag="htp")
            nc.tensor.transpose(out=htp[:], in_=h_t[:], identity=ident[:])
            ht = pool.tile([F, P], f32, tag="ht")
            nc.vector.tensor_copy(out=ht[:], in_=htp[:])
            op_t = psep.tile([P, F], f32, tag="op")
            nc.tensor.matmul(out=op_t[:], lhsT=ht[:], rhs=wmat_t[0:F, :],
                             start=True, stop=True)
            badd = pool.tile([P, F], f32, tag="badd")
            nc.vector.tensor_scalar(out=badd[:], in0=bv,
                                    scalar1=w1[:, 0:1], scalar2=None,
                                    op0=OP.mult)
            if g % 7 == 0:
                outs_sb = pool.tile([P, 7 * F], f32, tag="outs")
            nc.vector.tensor_tensor(
                out=outs_sb[:, (g % 7) * F:(g % 7 + 1) * F],
                in0=op_t[:], in1=badd[:], op=OP.add)
            if g % 7 == 6:
                g0 = g - 6
                nc.sync.dma_start(
                    out=out3[g0:g0 + 7].rearrange("g p f -> p g f"),
                    in_=outs_sb[:].rearrange("p (g f) -> p g f", f=F))


# ----------------------------------------------------------------------------
# host-side preparation (index plumbing only)
# ----------------------------------------------------------------------------

def prep_inputs(feature_a, feature_b, W, b, a_vec, edges, node_num_a,
                ncores=NCORES):
    fa = np.asarray(feature_a, np.float32)
    fb = np.asarray(feature_b, np.float32)
    W = np.asarray(W, np.float32)
    b = np.asarray(b, np.float32)
    a_vec = np.asarray(a_vec, np.float32).reshape(-1)
    edges = np.asarray(edges)
    NA = int(node_num_a)
    NB, Fdim = fb.shape
    assert Fdim == F and fa.shape[1] == F

    a1 = a_vec[:F]
    a2 = a_vec[F:]
    Wa2 = (W @ a2).astype(np.float32)
    ba2 = float(b @ a2)

    nodes_per_core = -(-NA // (ncores * P)) * P          # 6272
    Gc = nodes_per_core // P                             # 49
    NA_pad = nodes_per_core * ncores

    H = -(-NB // 256) * 128                              # 25088 (<= 32768)
    assert H <= 32768 and NB <= 2 * H
    fb_pad = np.zeros((2 * H, F), np.float32)
    fb_pad[:NB] = fb

    src = edges[:, 0].astype(np.int64)
    dst = edges[:, 1].astype(np.int64)
    core = src // nodes_per_core
    half = (dst >= H).astype(np.int64)
    dloc = dst - half * H
    sl_node = src - core * nodes_per_core
    g_all = sl_node // P
    sloc_all = sl_node % P

    # per (core, half, group) counts -> cross-core uniform chunk counts
    cell = (core * 2 + half) * Gc + g_all
    counts = np.bincount(cell, minlength=ncores * 2 * Gc) \
        .reshape(ncores, 2, Gc)
    nch = np.maximum(1, -(-counts.max(axis=0) // P))     # [2, Gc]

    # chunk id layout per half: group-major chunk streams
    chunk_of_g = [np.concatenate([[0], np.cumsum(nch[X])]) for X in (0, 1)]
    nch_tot = [int(nch[X].sum()) for X in (0, 1)]
    NBATCH = [-(-nch_tot[X] // CB) for X in (0, 1)]

    # per-half metadata (identical for all cores)
    meta = []
    for X in (0, 1):
        chunk_groups = np.full(NBATCH[X] * CB, -1, np.int64)
        chunk_groups[:nch_tot[X]] = np.repeat(np.arange(Gc), nch[X])
        batches = []
        for bi in range(NBATCH[X]):
            cg = chunk_groups[bi * CB:(bi + 1) * CB]
            runs = []
            cprev = 0
            for c in range(1, CB + 1):
                if c == CB or cg[c] != cg[cprev]:
                    if cg[cprev] >= 0:
                        runs.append((cprev, c, int(cg[cprev])))
                    cprev = c
            batches.append(dict(runs=runs))
        group_chunks = []
        for g in range(Gc):
            lo, hi = chunk_of_g[X][g], chunk_of_g[X][g + 1]
            group_chunks.append([(int(ci // CB), int(ci % CB))
                                 for ci in range(lo, hi)])
        meta.append(dict(batches=batches, group_chunks=group_chunks))

    iota8 = np.tile(np.arange(P, dtype=np.float32)[None, :], (P, CB))

    wvec = np.zeros((P, 3 * F), np.float32)
    wvec[:, 0:F] = a1[None, :]
    wvec[:, F:2 * F] = Wa2[None, :]
    wvec[:, 2 * F:3 * F] = b[None, :]

    in_maps = []
    for c in range(ncores):
        msk = core == c
        ehalf = half[msk]
        edloc = dloc[msk]
        eg = g_all[msk]
        esloc = sloc_all[msk]
        order = np.lexsort((esloc, eg, ehalf))
        ehalf, edloc, eg, esloc = (x[order] for x in
                                   (ehalf, edloc, eg, esloc))
        gidx_maps = []
        gidx2_maps = []
        sloc_maps = []
        for X in (0, 1):
            nslot = NBATCH[X] * CB * P
            dsl = np.zeros(nslot, np.int64)
            nsl = np.zeros(nslot, np.int64)
            ssl = np.full(nslot, -1.0, np.float32)
            selX = ehalf == X
            dX = edloc[selX]
            gX = eg[selX]
            sX = esloc[selX]
            cnts = np.bincount(gX, minlength=Gc)
            off = np.concatenate([[0], np.cumsum(cnts)])
            for g in range(Gc):
                n = int(cnts[g])
                base = int(chunk_of_g[X][g]) * P
                dsl[base:base + n] = dX[off[g]:off[g] + n]
                nsl[base:base + n] = g * P + sX[off[g]:off[g] + n]
                ssl[base:base + n] = sX[off[g]:off[g] + n]
            # pack gather indices: flat[i = c*128+e]; 16-wrap + 8x tile
            dslb = dsl.reshape(NBATCH[X], CB * P)
            nslb = nsl.reshape(NBATCH[X], CB * P)
            gmaps = np.zeros((NBATCH[X], P, CB * P // 16), np.int16)
            g2maps = np.zeros((NBATCH[X], P, CB * P // 16), np.int16)
            smaps = np.zeros((NBATCH[X], P, CB), np.float32)
            for bi in range(NBATCH[X]):
                sb = dslb[bi].reshape(CB * P // 16, 16).T.astype(np.int16)
                gmaps[bi] = np.tile(sb, (8, 1))
                sb2 = nslb[bi].reshape(CB * P // 16, 16).T.astype(np.int16)
                g2maps[bi] = np.tile(sb2, (8, 1))
                smaps[bi] = ssl[bi * CB * P:(bi + 1) * CB * P] \
                    .reshape(CB, P).T
            gidx_maps.append(gmaps)
            gidx2_maps.append(g2maps)
            sloc_maps.append(smaps)

        fa_core = np.zeros((nodes_per_core, F), np.float32)
        lo = c * nodes_per_core
        hi = min(lo + nodes_per_core, NA)
        fa_core[:hi - lo] = fa[lo:hi]
        fa_pk = np.ascontiguousarray(
            fa_core.reshape(Gc, P, F).transpose(1, 0, 2).reshape(P, Gc * F))

        in_maps.append(dict(
            fb_pad=fb_pad,
            fa_pk=fa_pk,
            gidxA=gidx_maps[0], gidxB=gidx_maps[1],
            gidx2A=gidx2_maps[0], gidx2B=gidx2_maps[1],
            slocA=sloc_maps[0], slocB=sloc_maps[1],
            iota8=iota8,
            wvec=wvec,
            wmat=np.ascontiguousarray(W),
        ))

    cfg = dict(Gc=Gc, H=H, ba2=ba2, meta=meta, NBATCH=NBATCH,
               nodes_per_core=nodes_per_core, NA=NA)
    return in_maps, cfg


def build_bass(cfg, ncores=NCORES):
    nc = bacc.Bacc("TRN2", target_bir_lowering=False, debug=False,
                   enable_asserts=False, num_devices=ncores,
                   num_swdge_queues=NQ)
    Gc, H = cfg["Gc"], cfg["H"]
    NBATCH = cfg["NBATCH"]
    ins = dict(
        fb_pad=nc.dram_tensor("fb_pad", [2 * H, F], f32,
                              kind="ExternalInput").ap(),
        fa_pk=nc.dram_tensor("fa_pk", [P, Gc * F], f32,
                             kind="ExternalInput").ap(),
        gidxA=nc.dram_tensor("gidxA", [NBATCH[0], P, CB * P // 16], i16,
                             kind="ExternalInput").ap(),
        gidxB=nc.dram_tensor("gidxB", [NBATCH[1], P, CB * P // 16], i16,
                             kind="ExternalInput").ap(),
        slocA=nc.dram_tensor("slocA", [NBATCH[0], P, CB], f32,
                             kind="ExternalInput").ap(),
        slocB=nc.dram_tensor("slocB", [NBATCH[1], P, CB], f32,
                             kind="ExternalInput").ap(),
        iota8=nc.dram_tensor("iota8", [P, CB * P], f32,
                             kind="ExternalInput").ap(),
        wvec=nc.dram_tensor("wvec", [P, 3 * F], f32,
                            kind="ExternalInput").ap(),
        wmat=nc.dram_tensor("wmat", [F, F], f32, kind="ExternalInput").ap(),
        e1tab=nc.dram_tensor("e1tab", [Gc * P, F], f32, kind="Internal").ap(),
        gidx2A=nc.dram_tensor("gidx2A", [NBATCH[0], P, CB * P // 16], i16,
                              kind="ExternalInput").ap(),
        gidx2B=nc.dram_tensor("gidx2B", [NBATCH[1], P, CB * P // 16], i16,
                              kind="ExternalInput").ap(),
    )
    outs = dict(
        out=nc.dram_tensor("out", [Gc * P, F], f32,
                           kind="ExternalOutput").ap(),
    )
    with tile.TileContext(nc) as tc:
        emit_program(tc, ins, outs, cfg)
    nc.compile()
    return nc


# ----------------------------------------------------------------------------
# entry point
# ----------------------------------------------------------------------------

def kernel(**inputs):
    from concourse import bass_utils

    in_maps, cfg = prep_inputs(**inputs)
    nc = build_bass(cfg)
    res = bass_utils.run_bass_kernel_spmd(
        nc, in_maps, core_ids=list(range(NCORES)))
    outs = [r["out"][:cfg["nodes_per_core"]] for r in res.results]
    full = np.concatenate(outs, axis=0)[:cfg["NA"]]
    return full.astype(np.float32)


# ----------------------------------------------------------------------------
# numpy emulation of the device program (for host-prep validation)
# ----------------------------------------------------------------------------

def emulate(in_maps, cfg):
    Gc, H = cfg["Gc"], cfg["H"]
    meta = cfg["meta"]
    ba2 = cfg["ba2"]
    outs = []
    for im in in_maps:
        fb_pad = im["fb_pad"]
        fa = im["fa_pk"].reshape(P, Gc, F).transpose(1, 0, 2) \
            .reshape(Gc * P, F)
        a1 = im["wvec"][0, 0:F]
        Wa2 = im["wvec"][0, F:2 * F]
        bvec = im["wvec"][0, 2 * F:3 * F]
        W = im["wmat"]
        e1 = fa @ a1 + ba2                       # [Gc*P]
        acc = np.zeros((Gc * P, F + 1), np.float64)
        for X, gname, g2name, sname in ((0, "gidxA", "gidx2A", "slocA"),
                                        (1, "gidxB", "gidx2B", "slocB")):
            gm, g2m, sm = im[gname], im[g2name], im[sname]
            NBx = gm.shape[0]
            for bi in range(NBx):
                # unpack idx: [128, CB*P//16] -> flat
                sb = gm[bi][:16]                 # [16, CB*P//16]
                flat = sb.T.reshape(-1).astype(np.int64)   # i = c*128+e
                didx = flat.reshape(CB, P)       # [c, e]
                nidx = g2m[bi][:16].T.reshape(-1).astype(np.int64) \
                    .reshape(CB, P)
                sl = sm[bi]                      # [e, c]
                rows = fb_pad[X * H + didx]      # [c, e, F]
                q = rows @ Wa2                   # [c, e]
                for (c0, c1, g) in meta[X]["batches"][bi]["runs"]:
                    for c in range(c0, c1):
                        sle = sl[:, c]
                        valid = sle >= 0
                        s = q[c] + e1[nidx[c]]
                        score = np.where(
                            s > 0, np.exp(s),
                            np.exp(0.1 * np.exp(np.minimum(s, 0)) - 0.1))
                        score = score * valid
                        onehot = (sle[:, None] ==
                                  np.arange(P)[None, :])   # [e, n]
                        acc[g * P:(g + 1) * P, 0:F] += \
                            onehot.T @ (score[:, None] * rows[c])
                        acc[g * P:(g + 1) * P, F] += onehot.T @ score
        den = acc[:, F]
        dsafe = np.where(den == 0, 1.0, den)
        h = acc[:, 0:F] / dsafe[:, None]
        o = h @ W + (den > 0)[:, None] * bvec[None, :]
        outs.append(o.astype(np.float32))
    full = np.concatenate(outs, 0)[:cfg["NA"]]
    return full


if __name__ == "__main__":
    np.random.seed(0)
    NA = NB = 50000
    E = 800000
    ins = dict(
        feature_a=np.random.randn(NA, F).astype(np.float32),
        feature_b=np.random.randn(NB, F).astype(np.float32),
        W=(np.random.randn(F, F) / 8).astype(np.float32),
        b=(np.random.randn(F) * 0.1).astype(np.float32),
        a_vec=(np.random.randn(2 * F, 1) * 0.05).astype(np.float32),
        edges=np.stack([np.random.randint(0, NA, E),
                        np.random.randint(0, NB, E)], 1).astype(np.int64),
        node_num_a=NA,
    )
    # numpy reference
    fa, fb = ins["feature_a"], ins["feature_b"]
    W, b_, av = ins["W"], ins["b"], ins["a_vec"].reshape(-1)
    src, dst = ins["edges"][:, 0], ins["edges"][:, 1]
    new_emb = fb @ W + b_
    s = (fa @ av[:F])[src] + (new_emb @ av[F:])[dst]
    score = np.exp(np.where(s > 0, s, 0.1 * (np.exp(np.minimum(s, 0)) - 1)))
    num = np.zeros((NA, F))
    np.add.at(num, src, score[:, None] * new_emb[dst])
    den = np.zeros(NA)
    np.add.at(den, src, score)
    dsafe = np.where(den == 0, 1, den)
    expected = num / dsafe[:, None]

    in_maps, cfg = prep_inputs(**ins)
    print("NBATCH:", cfg["NBATCH"], "slots:",
          sum(n * CB * P for n in cfg["NBATCH"]))
    got = emulate(in_maps, cfg)
    rel = np.linalg.norm(got - expected) / np.linalg.norm(expected)
    print("emulation rel err:", rel)


# revision 3
# speedup vs baseline: 4341.6965x; 1.0850x over previous
"""Trainium2 Bass kernel for AttentionAggregator (GNN message passing), v2.

Reference computation:
    new_emb = fb @ W + b
    s_e     = (fa @ a1)[src_e] + (new_emb @ a2)[dst_e]
    score_e = exp(elu(s_e, 0.1))
    out[n]  = (sum_{e: src_e=n} score_e * new_emb[dst_e]) / max(den[n], 1 if 0)

Reformulation (linearity of the segment sum):
    q_e   = fb[dst_e] @ (W @ a2)
    s_e   = (fa @ a1 + b @ a2)[src_e] + q_e
    G[n]  = sum_e score_e * fb[dst_e];  den[n] = sum_e score_e
    out[n]= (G[n] / den_safe[n]) @ W + 1[den>0] * b

Distribution: src nodes sharded contiguously across 8 cores (6272 rows
each); fb replicated.  No cross-core collective needed.

Device algorithm (per core):
  - Edges sorted by (dst-half, src).  Per src-group g (128 nodes) and
    dst-half X, edges are padded into chunks of 128 slots.  Chunk counts
    per (g, X) are cross-core uniform (max over cores) so one SPMD
    program serves all cores.
  - Chunks stream in batches of 8 (=1024 slots): one dma_gather per
    batch fetches fb[dst] f32 rows (256B) straight from the fb input,
    rotating over 4 SWDGE queues (~1.4 ns/row).
  - Per batch (vector/scalar engines): q = <row, W@a2>; one-hot
    onehot[e,n] = (srcloc[e]==n) via iota compare; e1 per edge via a
    second dma_gather from a device-built row-replicated e1 table;
    score = exp(elu(q+e1, 0.1)); rhs65 = [score*fb | score] in bf16.
  - Segment sum on the PE: per chunk, matmul(onehot^T @ rhs65)
    accumulates [128 nodes x 65] (G | den) in a PSUM bank; group-major
    consumption keeps only ~3 accumulator banks live.
  - Epilogue per group: divide by den_safe, transpose via PE, multiply
    by W, add (den>0)*b, DMA out.
"""

import sys

for _p in ("/opt/trn_rl_repo",):
    if _p not in sys.path:
        sys.path.insert(0, _p)

import numpy as np

import concourse.bass as bass
import concourse.bacc as bacc
import concourse.mybir as mybir
import concourse.tile as tile
from concourse.masks import make_identity

P = 128
F = 64
NCORES = 8
CB = 8            # chunks per gather batch (8*128 = 1024 idxs)
NQ = 4            # SWDGE queues

f32 = mybir.dt.float32
bf16 = mybir.dt.bfloat16
i16 = mybir.dt.int16
u8 = mybir.dt.uint8
AX = mybir.AxisListType
OP = mybir.AluOpType
ACTF = mybir.ActivationFunctionType


# ----------------------------------------------------------------------------
# device program
# ----------------------------------------------------------------------------

def emit_program(tc, ins, outs, cfg):
    nc = tc.nc
    abl = cfg.get("ablate", set())
    Gc = cfg["Gc"]                      # 49 src groups
    H = cfg["H"]                        # rows per dst half
    meta = cfg["meta"]                  # per-half batch/chunk metadata
    ba2 = float(cfg["ba2"])
    fb_pad = ins["fb_pad"]              # [2H, F] f32
    fa_pk = ins["fa_pk"]                # [P, Gc*F] f32 (fa[g*128+p, f])
    gidx = {0: ins["gidxA"], 1: ins["gidxB"]}    # [NB_X, P, CB*P//16] i16
    gidx2 = {0: ins["gidx2A"], 1: ins["gidx2B"]}  # same layout, src-local idx
    sloc = {0: ins["slocA"], 1: ins["slocB"]}    # [NB_X, P, CB] f32
    iota8 = ins["iota8"]                # [P, CB*P] f32 (each part: 8x iota128)
    wvec = ins["wvec"]                  # [P, 3F] f32: a1 | Wa2 | b rows
    wmat = ins["wmat"]                  # [F, F] f32
    out = outs["out"]                   # [Gc*P, F] f32

    halves = {0: fb_pad[0:H, :], 1: fb_pad[H:2 * H, :]}

    with (
        tc.tile_pool(name="const", bufs=1) as cpool,
        tc.tile_pool(name="work", bufs=3) as pool,
        tc.tile_pool(name="psum", bufs=3, space="PSUM") as psum,
        tc.tile_pool(name="psep", bufs=2, space="PSUM") as psep,
    ):
        # ---------------- constants -----------------------------------
        wvec_t = cpool.tile([P, 3 * F], f32)
        nc.sync.dma_start(out=wvec_t[:], in_=wvec)
        a1v = wvec_t[:, 0:F]
        w2v = wvec_t[:, F:2 * F]
        bv = wvec_t[:, 2 * F:3 * F]
        wmat_t = cpool.tile([P, F], f32)
        nc.sync.dma_start(out=wmat_t[0:F, :], in_=wmat)
        iota_t = cpool.tile([P, CB * P], f32)
        nc.sync.dma_start(out=iota_t[:], in_=iota8)
        ident = cpool.tile([P, P], f32)
        make_identity(nc, ident[:])
        zbias = cpool.tile([P, 1], f32)
        nc.vector.memset(zbias[:], 0.0)
        mbias = cpool.tile([P, 1], f32)
        nc.vector.memset(mbias[:], -0.1)

        # ---------------- e1 = fa @ a1 + ba2, replicated --------------
        fa_t = cpool.tile([P, Gc * F], f32)
        nc.sync.dma_start(out=fa_t[:], in_=fa_pk)
        fprod = cpool.tile([P, Gc * F], f32)
        nc.vector.tensor_tensor(
            out=fprod[:].rearrange("p (g f) -> p g f", f=F),
            in0=fa_t[:].rearrange("p (g f) -> p g f", f=F),
            in1=a1v[:, None, :].to_broadcast([P, Gc, F]), op=OP.mult)
        e1_all = cpool.tile([P, Gc], f32)
        nc.vector.tensor_reduce(
            out=e1_all[:],
            in_=fprod[:].rearrange("p (g f) -> p g f", f=F),
            axis=AX.X, op=OP.add)
        nc.vector.tensor_scalar(
            out=e1_all[:], in0=e1_all[:], scalar1=ba2, scalar2=None,
            op0=OP.add)
        # e1tab[g*128+p, 0:F] = e1[g*128+p] (row-replicated) in DRAM, so
        # per-edge e1 comes from a second dma_gather stream (idx = src-local)
        e1sb = cpool.tile([P, Gc * F], f32)
        nc.vector.tensor_scalar(
            out=e1sb[:].rearrange("p (g f) -> p g f", f=F),
            in0=e1_all[:, :, None].to_broadcast([P, Gc, F]),
            scalar1=0.0, scalar2=None, op0=OP.add)
        e1tab = ins["e1tab"]
        nc.sync.dma_start(
            out=e1tab.rearrange("(g p) f -> p g f", p=P),
            in_=e1sb[:].rearrange("p (g f) -> p g f", f=F))
        tc.strict_bb_all_engine_barrier()

        # ---------------- per-batch compute ---------------------------
        batch_tiles = {}
        gather_ctr = [0]

        def get_batch(X, b):
            key = (X, b)
            if key in batch_tiles:
                return batch_tiles[key]
            gi = pool.tile([P, CB * P // 16], i16, tag=f"gi{X}")
            nc.sync.dma_start(out=gi[:], in_=gidx[X][b])
            gi2 = pool.tile([P, CB * P // 16], i16, tag=f"gi2{X}")
            nc.sync.dma_start(out=gi2[:], in_=gidx2[X][b])
            sl = pool.tile([P, CB], f32, tag=f"sl{X}")
            nc.sync.dma_start(out=sl[:], in_=sloc[X][b])
            rows = pool.tile([P, CB * F], f32, tag=f"rows{X}", bufs=4)
            rows3 = rows[:].rearrange("p (c f) -> p c f", f=F)
            if "nogather" not in abl:
                nc.gpsimd.dma_gather(
                    out_ap=rows3, in_ap=halves[X], idxs_ap=gi[:],
                    num_idxs=CB * P, num_idxs_reg=CB * P, elem_size=F,
                    queue_num=gather_ctr[0] % NQ)
            gather_ctr[0] += 1

            # q[p, c] = <rows[p, c, :], Wa2>
            qprod = pool.tile([P, CB * F], f32, tag=f"qp{X}")
            nc.vector.tensor_tensor(
                out=qprod[:].rearrange("p (c f) -> p c f", f=F),
                in0=rows3,
                in1=w2v[:, None, :].to_broadcast([P, CB, F]), op=OP.mult)
            q_t = pool.tile([P, CB], f32, tag=f"q{X}")
            nc.vector.tensor_reduce(
                out=q_t[:],
                in_=qprod[:].rearrange("p (c f) -> p c f", f=F),
                axis=AX.X, op=OP.add)

            # onehot[e, c, n] = (sloc[e, c] == n)
            oh = pool.tile([P, CB * P], bf16, tag=f"oh{X}")
            oh3 = oh[:].rearrange("p (c n) -> p c n", n=P)
            nc.vector.tensor_tensor(
                out=oh3,
                in0=sl[:, :, None].to_broadcast([P, CB, P]),
                in1=iota_t[:].rearrange("p (c n) -> p c n", n=P),
                op=OP.is_equal)

            # e1 per edge via gather from e1tab (idx = src-local node)
            e1g = pool.tile([P, CB * F], f32, tag=f"e1g{X}", bufs=4)
            e1g3 = e1g[:].rearrange("p (c f) -> p c f", f=F)
            if "noe1" not in abl:
                nc.gpsimd.dma_gather(
                    out_ap=e1g3, in_ap=e1tab, idxs_ap=gi2[:],
                    num_idxs=CB * P, num_idxs_reg=CB * P, elem_size=F,
                    queue_num=gather_ctr[0] % NQ)
                gather_ctr[0] += 1

            # score = exp(elu(q + e1, 0.1))
            s_t = pool.tile([P, CB], f32, tag=f"s{X}")
            nc.vector.tensor_tensor(out=s_t[:], in0=q_t[:],
                                    in1=e1g3[:, :, 0], op=OP.add)
            t_t = pool.tile([P, CB], f32, tag=f"t{X}")
            nc.scalar.activation(t_t[:], s_t[:], ACTF.Exp,
                                 bias=zbias[:, 0:1], scale=1.0)
            u_t = pool.tile([P, CB], f32, tag=f"u{X}")
            nc.scalar.activation(u_t[:], t_t[:], ACTF.Exp,
                                 bias=mbias[:, 0:1], scale=0.1)
            m_t = pool.tile([P, CB], u8, tag=f"m{X}")
            nc.vector.tensor_scalar(
                out=m_t[:], in0=s_t[:], scalar1=0.0, scalar2=None,
                op0=OP.is_gt)
            nc.vector.copy_predicated(out=u_t[:], mask=m_t[:], data=t_t[:])

            # rhs65[e, c, :] = [score * fb_row | score]  (bf16)
            rhs = pool.tile([P, CB * (F + 1)], bf16, tag=f"rhs{X}")
            rhs3 = rhs[:].rearrange("p (c w) -> p c w", w=F + 1)
            nc.vector.tensor_tensor(
                out=rhs3[:, :, 0:F], in0=rows3,
                in1=u_t[:, :, None].to_broadcast([P, CB, F]), op=OP.mult)
            nc.vector.tensor_copy(out=rhs3[:, :, F], in_=u_t[:])

            res = (oh3, rhs3)
            batch_tiles[key] = res
            # keep the dict small: drop entries older than a few batches
            for k in list(batch_tiles):
                if k[0] == X and k[1] < b - 2:
                    del batch_tiles[k]
            return res

        # ---------------- group-major matmul + epilogue ---------------
        out3 = out.rearrange("(g p) f -> g p f", p=P)
        outs_sb = None
        for g in range(Gc):
            acc = psum.tile([P, F + 1], f32, tag="acc")
            first = True
            for X in (0, 1):
                for (b, c) in meta[X]["group_chunks"][g]:
                    oh3, rhs3 = get_batch(X, b)
                    last = (X == 1) and (b, c) == meta[1]["group_chunks"][g][-1]
                    if "nomm" not in abl:
                        nc.tensor.matmul(
                            out=acc[:], lhsT=oh3[:, c, :], rhs=rhs3[:, c, :],
                            start=first, stop=last)
                    first = False

            # epilogue for group g
            if "noep" in abl or "nomm" in abl:
                continue
            den = acc[:, F:F + 1]
            m0 = pool.tile([P, 1], f32, tag="m0")
            nc.vector.tensor_scalar(out=m0[:], in0=den, scalar1=0.0,
                                    scalar2=None, op0=OP.is_equal)
            w1 = pool.tile([P, 1], f32, tag="w1")
            nc.vector.tensor_scalar(out=w1[:], in0=den, scalar1=0.0,
                                    scalar2=None, op0=OP.is_gt)
            dsafe = pool.tile([P, 1], f32, tag="dsafe")
            nc.vector.tensor_tensor(out=dsafe[:], in0=den, in1=m0[:],
                                    op=OP.add)
            rec = pool.tile([P, 1], f32, tag="rec")
            nc.vector.reciprocal(rec[:], dsafe[:])
            h_t = pool.tile([P, F], f32, tag="h")
            nc.vector.tensor_scalar(out=h_t[:], in0=acc[:, 0:F],
                                    scalar1=rec[:, 0:1], scalar2=None,
                                    op0=OP.mult)
            htp = psep.tile([F, P], f32, tag="htp")
            nc.tensor.transpose(out=htp[:], in_=h_t[:], identity=ident[:])
            ht = pool.tile([F, P], f32, tag="ht")
            nc.vector.tensor_copy(out=ht[:], in_=htp[:])
            op_t = psep.tile([P, F], f32, tag="op")
            nc.tensor.matmul(out=op_t[:], lhsT=ht[:], rhs=wmat_t[0:F, :],
                             start=True, stop=True)
            badd = pool.tile([P, F], f32, tag="badd")
            nc.vector.tensor_scalar(out=badd[:], in0=bv,
                                    scalar1=w1[:, 0:1], scalar2=None,
                                    op0=OP.mult)
            if g % 7 == 0:
                outs_sb = pool.tile([P, 7 * F], f32, tag="outs")
            nc.vector.tensor_tensor(
                out=outs_sb[:, (g % 7) * F:(g % 7 + 1) * F],
                in0=op_t[:], in1=badd[:], op=OP.add)
            if g % 7 == 6:
                g0 = g - 6
                nc.sync.dma_start(
                    out=out3[g0:g0 + 7].rearrange("g p f -> p g f"),
                    in_=outs_sb[:].rearrange("p (g f) -> p g f", f=F))


# ----------------------------------------------------------------------------
# host-side preparation (index plumbing only)
# ----------------------------------------------------------------------------

def prep_inputs(feature_a, feature_b, W, b, a_vec, edges, node_num_a,
                ncores=NCORES):
    fa = np.asarray(feature_a, np.float32)
    fb = np.asarray(feature_b, np.float32)
    W = np.asarray(W, np.float32)
    b = np.asarray(b, np.float32)
    a_vec = np.asarray(a_vec, np.float32).reshape(-1)
    edges = np.asarray(edges)
    NA = int(node_num_a)
    NB, Fdim = fb.shape
    assert Fdim == F and fa.shape[1] == F

    a1 = a_vec[:F]
    a2 = a_vec[F:]
    Wa2 = (W @ a2).astype(np.float32)
    ba2 = float(b @ a2)

    nodes_per_core = -(-NA // (ncores * P)) * P          # 6272
    Gc = nodes_per_core // P                             # 49
    NA_pad = nodes_per_core * ncores

    H = -(-NB // 256) * 128                              # 25088 (<= 32768)
    assert H <= 32768 and NB <= 2 * H
    fb_pad = np.zeros((2 * H, F), np.float32)
    fb_pad[:NB] = fb

    src = edges[:, 0].astype(np.int64)
    dst = edges[:, 1].astype(np.int64)
    core = src // nodes_per_core
    half = (dst >= H).astype(np.int64)
    dloc = dst - half * H
    sl_node = src - core * nodes_per_core
    g_all = sl_node // P
    sloc_all = sl_node % P

    # per (core, half, group) counts -> cross-core uniform chunk counts
    cell = (core * 2 + half) * Gc + g_all
    counts = np.bincount(cell, minlength=ncores * 2 * Gc) \
        .reshape(ncores, 2, Gc)
    nch = np.maximum(1, -(-counts.max(axis=0) // P))     # [2, Gc]

    # chunk id layout per half: group-major chunk streams
    chunk_of_g = [np.concatenate([[0], np.cumsum(nch[X])]) for X in (0, 1)]
    nch_tot = [int(nch[X].sum()) for X in (0, 1)]
    NBATCH = [-(-nch_tot[X] // CB) for X in (0, 1)]

    # per-half metadata (identical for all cores)
    meta = []
    for X in (0, 1):
        chunk_groups = np.full(NBATCH[X] * CB, -1, np.int64)
        chunk_groups[:nch_tot[X]] = np.repeat(np.arange(Gc), nch[X])
        batches = []
        for bi in range(NBATCH[X]):
            cg = chunk_groups[bi * CB:(bi + 1) * CB]
            runs = []
            cprev = 0
            for c in range(1, CB + 1):
                if c == CB or cg[c] != cg[cprev]:
                    if cg[cprev] >= 0:
                        runs.append((cprev, c, int(cg[cprev])))
                    cprev = c
            batches.append(dict(runs=runs))
        group_chunks = []
        for g in range(Gc):
            lo, hi = chunk_of_g[X][g], chunk_of_g[X][g + 1]
            group_chunks.append([(int(ci // CB), int(ci % CB))
                                 for ci in range(lo, hi)])
        meta.append(dict(batches=batches, group_chunks=group_chunks))

    iota8 = np.tile(np.arange(P, dtype=np.float32)[None, :], (P, CB))

    wvec = np.zeros((P, 3 * F), np.float32)
    wvec[:, 0:F] = a1[None, :]
    wvec[:, F:2 * F] = Wa2[None, :]
    wvec[:, 2 * F:3 * F] = b[None, :]

    in_maps = []
    for c in range(ncores):
        msk = core == c
        ehalf = half[msk]
        edloc = dloc[msk]
        eg = g_all[msk]
        esloc = sloc_all[msk]
        order = np.lexsort((esloc, eg, ehalf))
        ehalf, edloc, eg, esloc = (x[order] for x in
                                   (ehalf, edloc, eg, esloc))
        gidx_maps = []
        gidx2_maps = []
        sloc_maps = []
        for X in (0, 1):
            nslot = NBATCH[X] * CB * P
            dsl = np.zeros(nslot, np.int64)
            nsl = np.zeros(nslot, np.int64)
            ssl = np.full(nslot, -1.0, np.float32)
            selX = ehalf == X
            dX = edloc[selX]
            gX = eg[selX]
            sX = esloc[selX]
            cnts = np.bincount(gX, minlength=Gc)
            off = np.concatenate([[0], np.cumsum(cnts)])
            for g in range(Gc):
                n = int(cnts[g])
                base = int(chunk_of_g[X][g]) * P
                dsl[base:base + n] = dX[off[g]:off[g] + n]
                nsl[base:base + n] = g * P + sX[off[g]:off[g] + n]
                ssl[base:base + n] = sX[off[g]:off[g] + n]
            # pack gather indices: flat[i = c*128+e]; 16-wrap + 8x tile
            dslb = dsl.reshape(NBATCH[X], CB * P)
            nslb = nsl.reshape(NBATCH[X], CB * P)
            gmaps = np.zeros((NBATCH[X], P, CB * P // 16), np.int16)
            g2maps = np.zeros((NBATCH[X], P, CB * P // 16), np.int16)
            smaps = np.zeros((NBATCH[X], P, CB), np.float32)
            for bi in range(NBATCH[X]):
                sb = dslb[bi].reshape(CB * P // 16, 16).T.astype(np.int16)
                gmaps[bi] = np.tile(sb, (8, 1))
                sb2 = nslb[bi].reshape(CB * P // 16, 16).T.astype(np.int16)
                g2maps[bi] = np.tile(sb2, (8, 1))
                smaps[bi] = ssl[bi * CB * P:(bi + 1) * CB * P] \
                    .reshape(CB, P).T
            gidx_maps.append(gmaps)
            gidx2_maps.append(g2maps)
            sloc_maps.append(smaps)

        fa_core = np.zeros((nodes_per_core, F), np.float32)
        lo = c * nodes_per_core
        hi = min(lo + nodes_per_core, NA)
        fa_core[:hi - lo] = fa[lo:hi]
        fa_pk = np.ascontiguousarray(
            fa_core.reshape(Gc, P, F).transpose(1, 0, 2).reshape(P, Gc * F))

        in_maps.append(dict(
            fb_pad=fb_pad,
            fa_pk=fa_pk,
            gidxA=gidx_maps[0], gidxB=gidx_maps[1],
            gidx2A=gidx2_maps[0], gidx2B=gidx2_maps[1],
            slocA=sloc_maps[0], slocB=sloc_maps[1],
            iota8=iota8,
            wvec=wvec,
            wmat=np.ascontiguousarray(W),
        ))

    cfg = dict(Gc=Gc, H=H, ba2=ba2, meta=meta, NBATCH=NBATCH,
               nodes_per_core=nodes_per_core, NA=NA)
    return in_maps, cfg


def build_bass(cfg, ncores=NCORES):
    nc = bacc.Bacc("TRN2", target_bir_lowering=False, debug=False,
                   enable_asserts=False, num_devices=ncores,
                   num_swdge_queues=NQ)
    Gc, H = cfg["Gc"], cfg["H"]
    NBATCH = cfg["NBATCH"]
    ins = dict(
        fb_pad=nc.dram_tensor("fb_pad", [2 * H, F], f32,
                              kind="ExternalInput").ap(),
        fa_pk=nc.dram_tensor("fa_pk", [P, Gc * F], f32,
                             kind="ExternalInput").ap(),
        gidxA=nc.dram_tensor("gidxA", [NBATCH[0], P, CB * P // 16], i16,
                             kind="ExternalInput").ap(),
        gidxB=nc.dram_tensor("gidxB", [NBATCH[1], P, CB * P // 16], i16,
                             kind="ExternalInput").ap(),
        slocA=nc.dram_tensor("slocA", [NBATCH[0], P, CB], f32,
                             kind="ExternalInput").ap(),
        slocB=nc.dram_tensor("slocB", [NBATCH[1], P, CB], f32,
                             kind="ExternalInput").ap(),
        iota8=nc.dram_tensor("iota8", [P, CB * P], f32,
                             kind="ExternalInput").ap(),
        wvec=nc.dram_tensor("wvec", [P, 3 * F], f32,
                            kind="ExternalInput").ap(),
        wmat=nc.dram_tensor("wmat", [F, F], f32, kind="ExternalInput").ap(),
        e1tab=nc.dram_tensor("e1tab", [Gc * P, F], f32, kind="Internal").ap(),
        gidx2A=nc.dram_tensor("gidx2A", [NBATCH[0], P, CB * P // 16], i16,
                              kind="ExternalInput").ap(),
        gidx2B=nc.dram_tensor("gidx2B", [NBATCH[1], P, CB * P // 16], i16,
                              kind="ExternalInput").ap(),
    )
    outs = dict(
        out=nc.dram_tensor("out", [Gc * P, F], f32,
                           kind="ExternalOutput").ap(),
    )
    with tile.TileContext(nc) as tc:
        emit_program(tc, ins, outs, cfg)
    nc.compile()
    return nc


# ----------------------------------------------------------------------------
# entry point
# ----------------------------------------------------------------------------

def kernel(**inputs):
    from concourse import bass_utils

    in_maps, cfg = prep_inputs(**inputs)
    nc = build_bass(cfg)
    res = bass_utils.run_bass_kernel_spmd(
        nc, in_maps, core_ids=list(range(NCORES)))
    outs = [r["out"][:cfg["nodes_per_core"]] for r in res.results]
    full = np.concatenate(outs, axis=0)[:cfg["NA"]]
    return full.astype(np.float32)


# ----------------------------------------------------------------------------
# numpy emulation of the device program (for host-prep validation)
# ----------------------------------------------------------------------------

def emulate(in_maps, cfg):
    Gc, H = cfg["Gc"], cfg["H"]
    meta = cfg["meta"]
    ba2 = cfg["ba2"]
    outs = []
    for im in in_maps:
        fb_pad = im["fb_pad"]
        fa = im["fa_pk"].reshape(P, Gc, F).transpose(1, 0, 2) \
            .reshape(Gc * P, F)
        a1 = im["wvec"][0, 0:F]
        Wa2 = im["wvec"][0, F:2 * F]
        bvec = im["wvec"][0, 2 * F:3 * F]
        W = im["wmat"]
        e1 = fa @ a1 + ba2                       # [Gc*P]
        acc = np.zeros((Gc * P, F + 1), np.float64)
        for X, gname, g2name, sname in ((0, "gidxA", "gidx2A", "slocA"),
                                        (1, "gidxB", "gidx2B", "slocB")):
            gm, g2m, sm = im[gname], im[g2name], im[sname]
            NBx = gm.shape[0]
            for bi in range(NBx):
                # unpack idx: [128, CB*P//16] -> flat
                sb = gm[bi][:16]                 # [16, CB*P//16]
                flat = sb.T.reshape(-1).astype(np.int64)   # i = c*128+e
                didx = flat.reshape(CB, P)       # [c, e]
                nidx = g2m[bi][:16].T.reshape(-1).astype(np.int64) \
                    .reshape(CB, P)
                sl = sm[bi]                      # [e, c]
                rows = fb_pad[X * H + didx]      # [c, e, F]
                q = rows @ Wa2                   # [c, e]
                for (c0, c1, g) in meta[X]["batches"][bi]["runs"]:
                    for c in range(c0, c1):
                        sle = sl[:, c]
                        valid = sle >= 0
                        s = q[c] + e1[nidx[c]]
                        score = np.where(
                            s > 0, np.exp(s),
                            np.exp(0.1 * np.exp(np.minimum(s, 0)) - 0.1))
                        score = score * valid
                        onehot = (sle[:, None] ==
                                  np.arange(P)[None, :])   # [e, n]
                        acc[g * P:(g + 1) * P, 0:F] += \
                            onehot.T @ (score[:, None] * rows[c])
                        acc[g * P:(g + 1) * P, F] += onehot.T @ score
        den = acc[:, F]
        dsafe = np.where(den == 0, 1.0, den)
        h = acc[:, 0:F] / dsafe[:, None]
        o = h @ W + (den > 0)[:, None] * bvec[None, :]
        outs.append(o.astype(np.float32))
    full = np.concatenate(outs, 0)[:cfg["NA"]]
    return full


if __name__ == "__main__":
    np.random.seed(0)
    NA = NB = 50000
    E = 800000
    ins = dict(
        feature_a=np.random.randn(NA, F).astype(np.float32),
        feature_b=np.random.randn(NB, F).astype(np.float32),
        W=(np.random.randn(F, F) / 8).astype(np.float32),
        b=(np.random.randn(F) * 0.1).astype(np.float32),
        a_vec=(np.random.randn(2 * F, 1) * 0.05).astype(np.float32),
        edges=np.stack([np.random.randint(0, NA, E),
                        np.random.randint(0, NB, E)], 1).astype(np.int64),
        node_num_a=NA,
    )
    # numpy reference
    fa, fb = ins["feature_a"], ins["feature_b"]
    W, b_, av = ins["W"], ins["b"], ins["a_vec"].reshape(-1)
    src, dst = ins["edges"][:, 0], ins["edges"][:, 1]
    new_emb = fb @ W + b_
    s = (fa @ av[:F])[src] + (new_emb @ av[F:])[dst]
    score = np.exp(np.where(s > 0, s, 0.1 * (np.exp(np.minimum(s, 0)) - 1)))
    num = np.zeros((NA, F))
    np.add.at(num, src, score[:, None] * new_emb[dst])
    den = np.zeros(NA)
    np.add.at(den, src, score)
    dsafe = np.where(den == 0, 1, den)
    expected = num / dsafe[:, None]

    in_maps, cfg = prep_inputs(**ins)
    print("NBATCH:", cfg["NBATCH"], "slots:",
          sum(n * CB * P for n in cfg["NBATCH"]))
    got = emulate(in_maps, cfg)
    rel = np.linalg.norm(got - expected) / np.linalg.norm(expected)
    print("emulation rel err:", rel)


# revision 4
# speedup vs baseline: 5318.9175x; 1.2251x over previous
"""Trainium2 Bass kernel for AttentionAggregator (GNN message passing), v2.

Reference computation:
    new_emb = fb @ W + b
    s_e     = (fa @ a1)[src_e] + (new_emb @ a2)[dst_e]
    score_e = exp(elu(s_e, 0.1))
    out[n]  = (sum_{e: src_e=n} score_e * new_emb[dst_e]) / max(den[n], 1 if 0)

Reformulation (linearity of the segment sum):
    q_e   = fb[dst_e] @ (W @ a2)
    s_e   = (fa @ a1 + b @ a2)[src_e] + q_e
    G[n]  = sum_e score_e * fb[dst_e];  den[n] = sum_e score_e
    out[n]= (G[n] / den_safe[n]) @ W + 1[den>0] * b

Distribution: src nodes sharded contiguously across 8 cores (6272 rows
each); fb replicated.  No cross-core collective needed.

Device algorithm (per core):
  - Edges sorted by (dst-half, src).  Per src-group g (128 nodes) and
    dst-half X, edges are padded into chunks of 128 slots.  Chunk counts
    per (g, X) are cross-core uniform (max over cores) so one SPMD
    program serves all cores.
  - Chunks stream in batches of 8 (=1024 slots): one dma_gather per
    batch fetches fb[dst] f32 rows (256B) straight from the fb input,
    rotating over 4 SWDGE queues (~1.4 ns/row).
  - Per batch (vector/scalar engines): q = <row, W@a2>; one-hot
    onehot[e,n] = (srcloc[e]==n) via iota compare; e1 per edge via
    onehot x replicated-e1 reduce; score = exp(elu(q+e1, 0.1)); rhs65 =
    [score*fb | score] in bf16.
  - Segment sum on the PE: per chunk, matmul(onehot^T @ rhs65)
    accumulates [128 nodes x 65] (G | den) in a PSUM bank; group-major
    consumption keeps only ~3 accumulator banks live.
  - Epilogue per group: divide by den_safe, transpose via PE, multiply
    by W, add (den>0)*b, DMA out.
"""

import sys

for _p in ("/opt/trn_rl_repo",):
    if _p not in sys.path:
        sys.path.insert(0, _p)

import numpy as np

import concourse.bass as bass
import concourse.bacc as bacc
import concourse.mybir as mybir
import concourse.tile as tile
from concourse.masks import make_identity

P = 128
F = 64
NCORES = 8
CB = 8            # chunks per gather batch (8*128 = 1024 idxs)
NQ = 4            # SWDGE queues
E1G = 0           # if >0, every E1G-th batch uses the e1 dma_gather path
                  # (measured slower than pure DVE select on this part)

f32 = mybir.dt.float32
bf16 = mybir.dt.bfloat16
i16 = mybir.dt.int16
u8 = mybir.dt.uint8
AX = mybir.AxisListType
OP = mybir.AluOpType
ACTF = mybir.ActivationFunctionType


# ----------------------------------------------------------------------------
# device program
# ----------------------------------------------------------------------------

def emit_program(tc, ins, outs, cfg):
    nc = tc.nc
    abl = cfg.get("ablate", set())
    Gc = cfg["Gc"]                      # 49 src groups
    H = cfg["H"]                        # rows per dst half
    meta = cfg["meta"]                  # per-half batch/chunk metadata
    ba2 = float(cfg["ba2"])
    fb_pad = ins["fb_pad"]              # [2H, F] f32
    fa_pk = ins["fa_pk"]                # [P, Gc*F] f32 (fa[g*128+p, f])
    gidx = {0: ins["gidxA"], 1: ins["gidxB"]}    # [NB_X, P, CB*P//16] i16
    gidx2 = ({0: ins["gidx2A"], 1: ins["gidx2B"]} if E1G else None)
    sloc = {0: ins["slocA"], 1: ins["slocB"]}    # [NB_X, P, CB] f32
    iota8 = ins["iota8"]                # [P, CB*P] f32 (each part: 8x iota128)
    wvec = ins["wvec"]                  # [P, 3F] f32: a1 | Wa2 | b rows
    wmat = ins["wmat"]                  # [F, F] f32
    out = outs["out"]                   # [Gc*P, F] f32

    halves = {0: fb_pad[0:H, :], 1: fb_pad[H:2 * H, :]}

    with (
        tc.tile_pool(name="const", bufs=1) as cpool,
        tc.tile_pool(name="work", bufs=3) as pool,
        tc.tile_pool(name="psum", bufs=3, space="PSUM") as psum,
        tc.tile_pool(name="psep", bufs=2, space="PSUM") as psep,
    ):
        # ---------------- constants -----------------------------------
        wvec_t = cpool.tile([P, 3 * F], f32)
        nc.sync.dma_start(out=wvec_t[:], in_=wvec)
        a1v = wvec_t[:, 0:F]
        w2v = wvec_t[:, F:2 * F]
        bv = wvec_t[:, 2 * F:3 * F]
        wmat_t = cpool.tile([P, F], f32)
        nc.sync.dma_start(out=wmat_t[0:F, :], in_=wmat)
        iota_t = cpool.tile([P, CB * P], f32)
        nc.sync.dma_start(out=iota_t[:], in_=iota8)
        ident = cpool.tile([P, P], f32)
        make_identity(nc, ident[:])
        zbias = cpool.tile([P, 1], f32)
        nc.vector.memset(zbias[:], 0.0)
        mbias = cpool.tile([P, 1], f32)
        nc.vector.memset(mbias[:], -0.1)

        # ---------------- e1 = fa @ a1 + ba2, replicated --------------
        fa_t = cpool.tile([P, Gc * F], f32)
        nc.sync.dma_start(out=fa_t[:], in_=fa_pk)
        fprod = cpool.tile([P, Gc * F], f32)
        nc.vector.tensor_tensor(
            out=fprod[:].rearrange("p (g f) -> p g f", f=F),
            in0=fa_t[:].rearrange("p (g f) -> p g f", f=F),
            in1=a1v[:, None, :].to_broadcast([P, Gc, F]), op=OP.mult)
        e1_all = cpool.tile([P, Gc], f32)
        nc.vector.tensor_reduce(
            out=e1_all[:],
            in_=fprod[:].rearrange("p (g f) -> p g f", f=F),
            axis=AX.X, op=OP.add)
        nc.vector.tensor_scalar(
            out=e1_all[:], in0=e1_all[:], scalar1=ba2, scalar2=None,
            op0=OP.add)
        # replicate e1 across partitions: e1rep[p, g*128+n] = e1[g*128+n].
        # Route through DRAM to flatten [128, Gc] -> one [1, Gc*128] row
        # (PE rhs base partition must be 0), then ones-matmul broadcast.
        ones1 = cpool.tile([1, P], f32)
        nc.vector.memset(ones1[:], 1.0)
        e1tp = psep.tile([Gc, P], f32, tag="htp")
        nc.tensor.transpose(out=e1tp[:], in_=e1_all[:], identity=ident[:])
        e1tps = cpool.tile([Gc, P], f32)
        nc.vector.tensor_copy(out=e1tps[:], in_=e1tp[:])
        e1d = ins["e1d"]
        nc.sync.dma_start(out=e1d, in_=e1tps[:])
        e1row = cpool.tile([1, Gc * P], f32)
        nc.sync.dma_start(out=e1row[0:1, :],
                          in_=e1d.rearrange("g p -> (g p)")[None, :])
        e1rep = cpool.tile([P, Gc * P], bf16)
        for k0 in range(0, Gc * P, 512):
            n = min(512, Gc * P - k0)
            erp = psep.tile([P, 512], f32, tag="erp", bufs=1)
            nc.tensor.matmul(out=erp[:, 0:n], lhsT=ones1[:],
                             rhs=e1row[0:1, k0:k0 + n], start=True, stop=True)
            nc.vector.tensor_copy(out=e1rep[:, k0:k0 + n], in_=erp[:, 0:n])
        if E1G:
            # e1tab[g*128+p, 0:F] = e1 (row-replicated) in DRAM for the
            # gather-based e1 path
            e1sb = cpool.tile([P, Gc * F], f32)
            nc.vector.tensor_scalar(
                out=e1sb[:].rearrange("p (g f) -> p g f", f=F),
                in0=e1_all[:, :, None].to_broadcast([P, Gc, F]),
                scalar1=0.0, scalar2=None, op0=OP.add)
            e1tab = ins["e1tab"]
            nc.sync.dma_start(
                out=e1tab.rearrange("(g p) f -> p g f", p=P),
                in_=e1sb[:].rearrange("p (g f) -> p g f", f=F))
            tc.strict_bb_all_engine_barrier()

        # ---------------- per-batch compute ---------------------------
        batch_tiles = {}
        gather_ctr = [0]

        def get_batch(X, b):
            key = (X, b)
            if key in batch_tiles:
                return batch_tiles[key]
            gi = pool.tile([P, CB * P // 16], i16, tag=f"gi{X}")
            nc.sync.dma_start(out=gi[:], in_=gidx[X][b])
            sl = pool.tile([P, CB], f32, tag=f"sl{X}")
            nc.sync.dma_start(out=sl[:], in_=sloc[X][b])
            rows = pool.tile([P, CB * F], f32, tag=f"rows{X}", bufs=4)
            rows3 = rows[:].rearrange("p (c f) -> p c f", f=F)
            if "nogather" not in abl:
                nc.gpsimd.dma_gather(
                    out_ap=rows3, in_ap=halves[X], idxs_ap=gi[:],
                    num_idxs=CB * P, num_idxs_reg=CB * P, elem_size=F,
                    queue_num=gather_ctr[0] % NQ)
            gather_ctr[0] += 1

            # q[p, c] = <rows[p, c, :], Wa2>
            qprod = pool.tile([P, CB * F], f32, tag=f"qp{X}")
            nc.vector.tensor_tensor(
                out=qprod[:].rearrange("p (c f) -> p c f", f=F),
                in0=rows3,
                in1=w2v[:, None, :].to_broadcast([P, CB, F]), op=OP.mult)
            q_t = pool.tile([P, CB], f32, tag=f"q{X}")
            nc.vector.tensor_reduce(
                out=q_t[:],
                in_=qprod[:].rearrange("p (c f) -> p c f", f=F),
                axis=AX.X, op=OP.add)

            # onehot[e, c, n] = (sloc[e, c] == n)
            oh = pool.tile([P, CB * P], bf16, tag=f"oh{X}")
            oh3 = oh[:].rearrange("p (c n) -> p c n", n=P)
            nc.vector.tensor_tensor(
                out=oh3,
                in0=sl[:, :, None].to_broadcast([P, CB, P]),
                in1=iota_t[:].rearrange("p (c n) -> p c n", n=P),
                op=OP.is_equal)

            # e1 per edge, two load-balanced mechanisms: every 4th batch
            # gathers e1 rows from e1tab (DMA path), the rest select via
            # onehot x e1rep reduce on DVE
            use_g = (E1G > 0 and gather_ctr[0] % E1G == 0
                     and "noe1" not in abl)
            if use_g:
                gi2 = pool.tile([P, CB * P // 16], i16, tag=f"gi2{X}")
                nc.sync.dma_start(out=gi2[:], in_=gidx2[X][b])
                e1g = pool.tile([P, CB * F], f32, tag=f"e1g{X}", bufs=4)
                e1g3 = e1g[:].rearrange("p (c f) -> p c f", f=F)
                nc.gpsimd.dma_gather(
                    out_ap=e1g3, in_ap=e1tab, idxs_ap=gi2[:],
                    num_idxs=CB * P, num_idxs_reg=CB * P, elem_size=F,
                    queue_num=gather_ctr[0] % NQ)
                gather_ctr[0] += 1
                ep_src = e1g3[:, :, 0]
            else:
                bm = meta[X]["batches"][b]
                ep_all = pool.tile([P, CB], f32, tag=f"ep{X}")
                eprod = pool.tile([P, CB * P], bf16, tag=f"eprod{X}")
                for (c0, c1, g) in (bm["runs"] if "noe1" not in abl else []):
                    nc.vector.tensor_tensor(
                        out=eprod[:].rearrange("p (c n) -> p c n", n=P)
                            [:, c0:c1, :],
                        in0=oh3[:, c0:c1, :],
                        in1=e1rep[:, g * P:(g + 1) * P][:, None, :]
                            .to_broadcast([P, c1 - c0, P]),
                        op=OP.mult)
                    nc.vector.tensor_reduce(
                        out=ep_all[:, c0:c1],
                        in_=eprod[:].rearrange("p (c n) -> p c n", n=P)
                            [:, c0:c1, :],
                        axis=AX.X, op=OP.add)
                if "noe1" in abl:
                    nc.vector.memset(ep_all[:], 0.0)
                ep_src = ep_all[:]

            # score = exp(elu(q + e1, 0.1))
            s_t = pool.tile([P, CB], f32, tag=f"s{X}")
            nc.vector.tensor_tensor(out=s_t[:], in0=q_t[:],
                                    in1=ep_src, op=OP.add)
            t_t = pool.tile([P, CB], f32, tag=f"t{X}")
            nc.scalar.activation(t_t[:], s_t[:], ACTF.Exp,
                                 bias=zbias[:, 0:1], scale=1.0)
            u_t = pool.tile([P, CB], f32, tag=f"u{X}")
            nc.scalar.activation(u_t[:], t_t[:], ACTF.Exp,
                                 bias=mbias[:, 0:1], scale=0.1)
            m_t = pool.tile([P, CB], u8, tag=f"m{X}")
            nc.vector.tensor_scalar(
                out=m_t[:], in0=s_t[:], scalar1=0.0, scalar2=None,
                op0=OP.is_gt)
            nc.vector.copy_predicated(out=u_t[:], mask=m_t[:], data=t_t[:])

            # rhs65[e, c, :] = [score * fb_row | score]  (bf16)
            rhs = pool.tile([P, CB * (F + 1)], bf16, tag=f"rhs{X}")
            rhs3 = rhs[:].rearrange("p (c w) -> p c w", w=F + 1)
            nc.vector.tensor_tensor(
                out=rhs3[:, :, 0:F], in0=rows3,
                in1=u_t[:, :, None].to_broadcast([P, CB, F]), op=OP.mult)
            nc.vector.tensor_copy(out=rhs3[:, :, F], in_=u_t[:])

            res = (oh3, rhs3)
            batch_tiles[key] = res
            # keep the dict small: drop entries older than a few batches
            for k in list(batch_tiles):
                if k[0] == X and k[1] < b - 2:
                    del batch_tiles[k]
            return res

        # ---------------- group-major matmul + epilogue ---------------
        out3 = out.rearrange("(g p) f -> g p f", p=P)
        outs_sb = None
        for g in range(Gc):
            acc = psum.tile([P, F + 1], f32, tag="acc")
            first = True
            for X in (0, 1):
                for (b, c) in meta[X]["group_chunks"][g]:
                    oh3, rhs3 = get_batch(X, b)
                    last = (X == 1) and (b, c) == meta[1]["group_chunks"][g][-1]
                    if "nomm" not in abl:
                        nc.tensor.matmul(
                            out=acc[:], lhsT=oh3[:, c, :], rhs=rhs3[:, c, :],
                            start=first, stop=last)
                    first = False

            # epilogue for group g
            if "noep" in abl or "nomm" in abl:
                continue
            den = acc[:, F:F + 1]
            m0 = pool.tile([P, 1], f32, tag="m0")
            nc.vector.tensor_scalar(out=m0[:], in0=den, scalar1=0.0,
                                    scalar2=None, op0=OP.is_equal)
            w1 = pool.tile([P, 1], f32, tag="w1")
            nc.vector.tensor_scalar(out=w1[:], in0=den, scalar1=0.0,
                                    scalar2=None, op0=OP.is_gt)
            dsafe = pool.tile([P, 1], f32, tag="dsafe")
            nc.vector.tensor_tensor(out=dsafe[:], in0=den, in1=m0[:],
                                    op=OP.add)
            rec = pool.tile([P, 1], f32, tag="rec")
            nc.vector.reciprocal(rec[:], dsafe[:])
            h_t = pool.tile([P, F], f32, tag="h")
            nc.vector.tensor_scalar(out=h_t[:], in0=acc[:, 0:F],
                                    scalar1=rec[:, 0:1], scalar2=None,
                                    op0=OP.mult)
            htp = psep.tile([F, P], f32, tag="htp")
            nc.tensor.transpose(out=htp[:], in_=h_t[:], identity=ident[:])
            ht = pool.tile([F, P], f32, tag="ht")
            nc.vector.tensor_copy(out=ht[:], in_=htp[:])
            op_t = psep.tile([P, F], f32, tag="op")
            nc.tensor.matmul(out=op_t[:], lhsT=ht[:], rhs=wmat_t[0:F, :],
                             start=True, stop=True)
            badd = pool.tile([P, F], f32, tag="badd")
            nc.vector.tensor_scalar(out=badd[:], in0=bv,
                                    scalar1=w1[:, 0:1], scalar2=None,
                                    op0=OP.mult)
            if g % 7 == 0:
                outs_sb = pool.tile([P, 7 * F], f32, tag="outs")
            nc.vector.tensor_tensor(
                out=outs_sb[:, (g % 7) * F:(g % 7 + 1) * F],
                in0=op_t[:], in1=badd[:], op=OP.add)
            if g % 7 == 6:
                g0 = g - 6
                nc.sync.dma_start(
                    out=out3[g0:g0 + 7].rearrange("g p f -> p g f"),
                    in_=outs_sb[:].rearrange("p (g f) -> p g f", f=F))


# ----------------------------------------------------------------------------
# host-side preparation (index plumbing only)
# ----------------------------------------------------------------------------

def prep_inputs(feature_a, feature_b, W, b, a_vec, edges, node_num_a,
                ncores=NCORES):
    fa = np.asarray(feature_a, np.float32)
    fb = np.asarray(feature_b, np.float32)
    W = np.asarray(W, np.float32)
    b = np.asarray(b, np.float32)
    a_vec = np.asarray(a_vec, np.float32).reshape(-1)
    edges = np.asarray(edges)
    NA = int(node_num_a)
    NB, Fdim = fb.shape
    assert Fdim == F and fa.shape[1] == F

    a1 = a_vec[:F]
    a2 = a_vec[F:]
    Wa2 = (W @ a2).astype(np.float32)
    ba2 = float(b @ a2)

    nodes_per_core = -(-NA // (ncores * P)) * P          # 6272
    Gc = nodes_per_core // P                             # 49
    NA_pad = nodes_per_core * ncores

    H = -(-NB // 256) * 128                              # 25088 (<= 32768)
    assert H <= 32768 and NB <= 2 * H
    fb_pad = np.zeros((2 * H, F), np.float32)
    fb_pad[:NB] = fb

    src = edges[:, 0].astype(np.int64)
    dst = edges[:, 1].astype(np.int64)
    core = src // nodes_per_core
    half = (dst >= H).astype(np.int64)
    dloc = dst - half * H
    sl_node = src - core * nodes_per_core
    g_all = sl_node // P
    sloc_all = sl_node % P

    # per (core, half, group) counts -> cross-core uniform chunk counts
    cell = (core * 2 + half) * Gc + g_all
    counts = np.bincount(cell, minlength=ncores * 2 * Gc) \
        .reshape(ncores, 2, Gc)
    nch = np.maximum(1, -(-counts.max(axis=0) // P))     # [2, Gc]

    # chunk id layout per half: group-major chunk streams
    chunk_of_g = [np.concatenate([[0], np.cumsum(nch[X])]) for X in (0, 1)]
    nch_tot = [int(nch[X].sum()) for X in (0, 1)]
    NBATCH = [-(-nch_tot[X] // CB) for X in (0, 1)]

    # per-half metadata (identical for all cores)
    meta = []
    for X in (0, 1):
        chunk_groups = np.full(NBATCH[X] * CB, -1, np.int64)
        chunk_groups[:nch_tot[X]] = np.repeat(np.arange(Gc), nch[X])
        batches = []
        for bi in range(NBATCH[X]):
            cg = chunk_groups[bi * CB:(bi + 1) * CB]
            runs = []
            cprev = 0
            for c in range(1, CB + 1):
                if c == CB or cg[c] != cg[cprev]:
                    if cg[cprev] >= 0:
                        runs.append((cprev, c, int(cg[cprev])))
                    cprev = c
            batches.append(dict(runs=runs))
        group_chunks = []
        for g in range(Gc):
            lo, hi = chunk_of_g[X][g], chunk_of_g[X][g + 1]
            group_chunks.append([(int(ci // CB), int(ci % CB))
                                 for ci in range(lo, hi)])
        meta.append(dict(batches=batches, group_chunks=group_chunks))

    iota8 = np.tile(np.arange(P, dtype=np.float32)[None, :], (P, CB))

    wvec = np.zeros((P, 3 * F), np.float32)
    wvec[:, 0:F] = a1[None, :]
    wvec[:, F:2 * F] = Wa2[None, :]
    wvec[:, 2 * F:3 * F] = b[None, :]

    in_maps = []
    for c in range(ncores):
        msk = core == c
        ehalf = half[msk]
        edloc = dloc[msk]
        eg = g_all[msk]
        esloc = sloc_all[msk]
        order = np.lexsort((esloc, eg, ehalf))
        ehalf, edloc, eg, esloc = (x[order] for x in
                                   (ehalf, edloc, eg, esloc))
        gidx_maps = []
        gidx2_maps = []
        sloc_maps = []
        for X in (0, 1):
            nslot = NBATCH[X] * CB * P
            dsl = np.zeros(nslot, np.int64)
            nsl = np.zeros(nslot, np.int64)
            ssl = np.full(nslot, -1.0, np.float32)
            selX = ehalf == X
            dX = edloc[selX]
            gX = eg[selX]
            sX = esloc[selX]
            cnts = np.bincount(gX, minlength=Gc)
            off = np.concatenate([[0], np.cumsum(cnts)])
            for g in range(Gc):
                n = int(cnts[g])
                base = int(chunk_of_g[X][g]) * P
                dsl[base:base + n] = dX[off[g]:off[g] + n]
                nsl[base:base + n] = g * P + sX[off[g]:off[g] + n]
                ssl[base:base + n] = sX[off[g]:off[g] + n]
            # pack gather indices: flat[i = c*128+e]; 16-wrap + 8x tile
            dslb = dsl.reshape(NBATCH[X], CB * P)
            nslb = nsl.reshape(NBATCH[X], CB * P)
            gmaps = np.zeros((NBATCH[X], P, CB * P // 16), np.int16)
            g2maps = np.zeros((NBATCH[X], P, CB * P // 16), np.int16)
            smaps = np.zeros((NBATCH[X], P, CB), np.float32)
            for bi in range(NBATCH[X]):
                sb = dslb[bi].reshape(CB * P // 16, 16).T.astype(np.int16)
                gmaps[bi] = np.tile(sb, (8, 1))
                sb2 = nslb[bi].reshape(CB * P // 16, 16).T.astype(np.int16)
                g2maps[bi] = np.tile(sb2, (8, 1))
                smaps[bi] = ssl[bi * CB * P:(bi + 1) * CB * P] \
                    .reshape(CB, P).T
            gidx_maps.append(gmaps)
            gidx2_maps.append(g2maps)
            sloc_maps.append(smaps)

        fa_core = np.zeros((nodes_per_core, F), np.float32)
        lo = c * nodes_per_core
        hi = min(lo + nodes_per_core, NA)
        fa_core[:hi - lo] = fa[lo:hi]
        fa_pk = np.ascontiguousarray(
            fa_core.reshape(Gc, P, F).transpose(1, 0, 2).reshape(P, Gc * F))

        in_maps.append(dict(
            fb_pad=fb_pad,
            fa_pk=fa_pk,
            gidxA=gidx_maps[0], gidxB=gidx_maps[1],
            **(dict(gidx2A=gidx2_maps[0], gidx2B=gidx2_maps[1])
               if E1G else {}),
            slocA=sloc_maps[0], slocB=sloc_maps[1],
            iota8=iota8,
            wvec=wvec,
            wmat=np.ascontiguousarray(W),
        ))

    cfg = dict(Gc=Gc, H=H, ba2=ba2, meta=meta, NBATCH=NBATCH,
               nodes_per_core=nodes_per_core, NA=NA)
    return in_maps, cfg


def build_bass(cfg, ncores=NCORES):
    nc = bacc.Bacc("TRN2", target_bir_lowering=False, debug=False,
                   enable_asserts=False, num_devices=ncores,
                   num_swdge_queues=NQ)
    Gc, H = cfg["Gc"], cfg["H"]
    NBATCH = cfg["NBATCH"]
    ins = dict(
        fb_pad=nc.dram_tensor("fb_pad", [2 * H, F], f32,
                              kind="ExternalInput").ap(),
        fa_pk=nc.dram_tensor("fa_pk", [P, Gc * F], f32,
                             kind="ExternalInput").ap(),
        gidxA=nc.dram_tensor("gidxA", [NBATCH[0], P, CB * P // 16], i16,
                             kind="ExternalInput").ap(),
        gidxB=nc.dram_tensor("gidxB", [NBATCH[1], P, CB * P // 16], i16,
                             kind="ExternalInput").ap(),
        slocA=nc.dram_tensor("slocA", [NBATCH[0], P, CB], f32,
                             kind="ExternalInput").ap(),
        slocB=nc.dram_tensor("slocB", [NBATCH[1], P, CB], f32,
                             kind="ExternalInput").ap(),
        iota8=nc.dram_tensor("iota8", [P, CB * P], f32,
                             kind="ExternalInput").ap(),
        wvec=nc.dram_tensor("wvec", [P, 3 * F], f32,
                            kind="ExternalInput").ap(),
        wmat=nc.dram_tensor("wmat", [F, F], f32, kind="ExternalInput").ap(),
        e1d=nc.dram_tensor("e1d", [Gc, P], f32, kind="Internal").ap(),

    )
    outs = dict(
        out=nc.dram_tensor("out", [Gc * P, F], f32,
                           kind="ExternalOutput").ap(),
    )
    with tile.TileContext(nc) as tc:
        emit_program(tc, ins, outs, cfg)
    nc.compile()
    return nc


# ----------------------------------------------------------------------------
# entry point
# ----------------------------------------------------------------------------

def kernel(**inputs):
    from concourse import bass_utils

    in_maps, cfg = prep_inputs(**inputs)
    nc = build_bass(cfg)
    res = bass_utils.run_bass_kernel_spmd(
        nc, in_maps, core_ids=list(range(NCORES)))
    outs = [r["out"][:cfg["nodes_per_core"]] for r in res.results]
    full = np.concatenate(outs, axis=0)[:cfg["NA"]]
    return full.astype(np.float32)


# ----------------------------------------------------------------------------
# numpy emulation of the device program (for host-prep validation)
# ----------------------------------------------------------------------------

def emulate(in_maps, cfg):
    Gc, H = cfg["Gc"], cfg["H"]
    meta = cfg["meta"]
    ba2 = cfg["ba2"]
    outs = []
    for im in in_maps:
        fb_pad = im["fb_pad"]
        fa = im["fa_pk"].reshape(P, Gc, F).transpose(1, 0, 2) \
            .reshape(Gc * P, F)
        a1 = im["wvec"][0, 0:F]
        Wa2 = im["wvec"][0, F:2 * F]
        bvec = im["wvec"][0, 2 * F:3 * F]
        W = im["wmat"]
        e1 = fa @ a1 + ba2                       # [Gc*P]
        acc = np.zeros((Gc * P, F + 1), np.float64)
        for X, gname, sname in ((0, "gidxA", "slocA"), (1, "gidxB", "slocB")):
            gm, sm = im[gname], im[sname]
            NBx = gm.shape[0]
            for bi in range(NBx):
                # unpack idx: [128, CB*P//16] -> flat
                sb = gm[bi][:16]                 # [16, CB*P//16]
                flat = sb.T.reshape(-1).astype(np.int64)   # i = c*128+e
                didx = flat.reshape(CB, P)       # [c, e]
                sl = sm[bi]                      # [e, c]
                rows = fb_pad[X * H + didx]      # [c, e, F]
                q = rows @ Wa2                   # [c, e]
                for (c0, c1, g) in meta[X]["batches"][bi]["runs"]:
                    for c in range(c0, c1):
                        sle = sl[:, c]
                        valid = sle >= 0
                        s = q[c] + e1[g * P + (sle.astype(np.int64) % P)]
                        score = np.where(
                            s > 0, np.exp(s),
                            np.exp(0.1 * np.exp(np.minimum(s, 0)) - 0.1))
                        score = score * valid
                        onehot = (sle[:, None] ==
                                  np.arange(P)[None, :])   # [e, n]
                        acc[g * P:(g + 1) * P, 0:F] += \
                            onehot.T @ (score[:, None] * rows[c])
                        acc[g * P:(g + 1) * P, F] += onehot.T @ score
        den = acc[:, F]
        dsafe = np.where(den == 0, 1.0, den)
        h = acc[:, 0:F] / dsafe[:, None]
        o = h @ W + (den > 0)[:, None] * bvec[None, :]
        outs.append(o.astype(np.float32))
    full = np.concatenate(outs, 0)[:cfg["NA"]]
    return full


if __name__ == "__main__":
    np.random.seed(0)
    NA = NB = 50000
    E = 800000
    ins = dict(
        feature_a=np.random.randn(NA, F).astype(np.float32),
        feature_b=np.random.randn(NB, F).astype(np.float32),
        W=(np.random.randn(F, F) / 8).astype(np.float32),
        b=(np.random.randn(F) * 0.1).astype(np.float32),
        a_vec=(np.random.randn(2 * F, 1) * 0.05).astype(np.float32),
        edges=np.stack([np.random.randint(0, NA, E),
                        np.random.randint(0, NB, E)], 1).astype(np.int64),
        node_num_a=NA,
    )
    # numpy reference
    fa, fb = ins["feature_a"], ins["feature_b"]
    W, b_, av = ins["W"], ins["b"], ins["a_vec"].reshape(-1)
    src, dst = ins["edges"][:, 0], ins["edges"][:, 1]
    new_emb = fb @ W + b_
    s = (fa @ av[:F])[src] + (new_emb @ av[F:])[dst]
    score = np.exp(np.where(s > 0, s, 0.1 * (np.exp(np.minimum(s, 0)) - 1)))
    num = np.zeros((NA, F))
    np.add.at(num, src, score[:, None] * new_emb[dst])
    den = np.zeros(NA)
    np.add.at(den, src, score)
    dsafe = np.where(den == 0, 1, den)
    expected = num / dsafe[:, None]

    in_maps, cfg = prep_inputs(**ins)
    print("NBATCH:", cfg["NBATCH"], "slots:",
          sum(n * CB * P for n in cfg["NBATCH"]))
    got = emulate(in_maps, cfg)
    rel = np.linalg.norm(got - expected) / np.linalg.norm(expected)
    print("emulation rel err:", rel)


# revision 5
# speedup vs baseline: 7207.5710x; 1.3551x over previous
"""Trainium2 Bass kernel for AttentionAggregator (GNN message passing), v2.

Reference computation:
    new_emb = fb @ W + b
    s_e     = (fa @ a1)[src_e] + (new_emb @ a2)[dst_e]
    score_e = exp(elu(s_e, 0.1))
    out[n]  = (sum_{e: src_e=n} score_e * new_emb[dst_e]) / max(den[n], 1 if 0)

Reformulation (linearity of the segment sum):
    q_e   = fb[dst_e] @ (W @ a2)
    s_e   = (fa @ a1 + b @ a2)[src_e] + q_e
    G[n]  = sum_e score_e * fb[dst_e];  den[n] = sum_e score_e
    out[n]= (G[n] / den_safe[n]) @ W + 1[den>0] * b

Distribution: src nodes sharded contiguously across 8 cores (6272 rows
each); fb replicated.  No cross-core collective needed.

Device algorithm (per core):
  - Edges sorted by (dst-half, src).  Per src-group g (128 nodes) and
    dst-half X, edges are padded into chunks of 128 slots.  Chunk counts
    per (g, X) are cross-core uniform (max over cores) so one SPMD
    program serves all cores.
  - Chunks stream in batches of 8 (=1024 slots): one dma_gather per
    batch fetches fb[dst] f32 rows (256B) straight from the fb input,
    rotating over 4 SWDGE queues (~1.4 ns/row).
  - Per batch (vector/scalar engines): q = <row, W@a2>; one-hot
    onehot[e,n] = (srcloc[e]==n) via iota compare; e1 per edge via
    onehot x replicated-e1 reduce; score = exp(elu(q+e1, 0.1)); rhs65 =
    [score*fb | score] in bf16.
  - Segment sum on the PE: per chunk, matmul(onehot^T @ rhs65)
    accumulates [128 nodes x 65] (G | den) in a PSUM bank; group-major
    consumption keeps only ~3 accumulator banks live.
  - Epilogue per group: divide by den_safe, transpose via PE, multiply
    by W, add (den>0)*b, DMA out.
"""

import sys

for _p in ("/opt/trn_rl_repo",):
    if _p not in sys.path:
        sys.path.insert(0, _p)

import numpy as np

import concourse.bass as bass
import concourse.bacc as bacc
import concourse.mybir as mybir
import concourse.tile as tile
from concourse.masks import make_identity

P = 128
F = 64
NCORES = 8
CB = 16           # chunks per batch (two 1024-idx dma_gathers per batch)
NQ = 4            # SWDGE queues
E1G = 0           # if >0, every E1G-th batch uses the e1 dma_gather path
                  # (measured slower than pure DVE select on this part)

f32 = mybir.dt.float32
bf16 = mybir.dt.bfloat16
i16 = mybir.dt.int16
u8 = mybir.dt.uint8
AX = mybir.AxisListType
OP = mybir.AluOpType
ACTF = mybir.ActivationFunctionType


# ----------------------------------------------------------------------------
# device program
# ----------------------------------------------------------------------------

def emit_program(tc, ins, outs, cfg):
    nc = tc.nc
    abl = cfg.get("ablate", set())
    Gc = cfg["Gc"]                      # 49 src groups
    H = cfg["H"]                        # rows per dst half
    meta = cfg["meta"]                  # per-half batch/chunk metadata
    ba2 = float(cfg["ba2"])
    NBATCH = cfg["NBATCH"]
    fb_pad = ins["fb_pad"]              # [2H, F] f32
    consts = ins["consts"]              # [P, CW]: iota8 | wvec | wmat | fa_pk
    gidx_all = ins["gidx_all"]          # [NB0+NB1, P, CB*P//16] i16
    sloc_all = ins["sloc_all"]          # [NB0+NB1, P, CB] f32
    bofs = {0: 0, 1: NBATCH[0]}
    gidx2 = ({0: ins["gidx2A"], 1: ins["gidx2B"]} if E1G else None)
    out = outs["out"]                   # [Gc*P, F] f32

    halves = {0: fb_pad[0:H, :], 1: fb_pad[H:2 * H, :]}

    with (
        tc.tile_pool(name="const", bufs=1) as cpool,
        tc.tile_pool(name="work", bufs=3) as pool,
        tc.tile_pool(name="psum", bufs=3, space="PSUM") as psum,
        tc.tile_pool(name="psep", bufs=2, space="PSUM") as psep,
    ):
        # ---------------- constants (one block load) -------------------
        CW = CB * P + 3 * F + F + Gc * F
        cbig = cpool.tile([P, CW], f32)
        nc.sync.dma_start(out=cbig[:], in_=consts)
        o_iota, o_wvec = 0, CB * P
        o_wmat, o_fa = CB * P + 3 * F, CB * P + 3 * F + F
        iota_t = cbig[:, o_iota:o_iota + CB * P]
        a1v = cbig[:, o_wvec:o_wvec + F]
        w2v = cbig[:, o_wvec + F:o_wvec + 2 * F]
        bv = cbig[:, o_wvec + 2 * F:o_wvec + 3 * F]
        wmat_t = cbig[:, o_wmat:o_wmat + F]
        fa_t = cbig[:, o_fa:o_fa + Gc * F]
        ident = cpool.tile([P, P], f32)
        make_identity(nc, ident[:])
        zbias = cpool.tile([P, 1], f32)
        nc.vector.memset(zbias[:], 0.0)
        mbias = cpool.tile([P, 1], f32)
        nc.vector.memset(mbias[:], -0.1)

        # ---------------- e1 = fa @ a1 + ba2, replicated --------------
        fprod = cpool.tile([P, Gc * F], f32)
        nc.vector.tensor_tensor(
            out=fprod[:].rearrange("p (g f) -> p g f", f=F),
            in0=fa_t.rearrange("p (g f) -> p g f", f=F),
            in1=a1v[:, None, :].to_broadcast([P, Gc, F]), op=OP.mult)
        e1_all = cpool.tile([P, Gc], f32)
        nc.vector.tensor_reduce(
            out=e1_all[:],
            in_=fprod[:].rearrange("p (g f) -> p g f", f=F),
            axis=AX.X, op=OP.add)
        nc.vector.tensor_scalar(
            out=e1_all[:], in0=e1_all[:], scalar1=ba2, scalar2=None,
            op0=OP.add)
        # replicate e1 across partitions: e1rep[p, g*128+n] = e1[g*128+n].
        # Route through DRAM to flatten [128, Gc] -> one [1, Gc*128] row
        # (PE rhs base partition must be 0), then ones-matmul broadcast.
        ones1 = cpool.tile([1, P], f32)
        nc.vector.memset(ones1[:], 1.0)
        e1tp = psep.tile([Gc, P], f32, tag="htp")
        nc.tensor.transpose(out=e1tp[:], in_=e1_all[:], identity=ident[:])
        e1tps = cpool.tile([Gc, P], f32)
        nc.vector.tensor_copy(out=e1tps[:], in_=e1tp[:])
        e1d = ins["e1d"]
        nc.sync.dma_start(out=e1d, in_=e1tps[:])
        e1row = cpool.tile([1, Gc * P], f32)
        nc.sync.dma_start(out=e1row[0:1, :],
                          in_=e1d.rearrange("g p -> (g p)")[None, :])
        e1rep = cpool.tile([P, Gc * P], bf16)
        for k0 in range(0, Gc * P, 512):
            n = min(512, Gc * P - k0)
            erp = psep.tile([P, 512], f32, tag="erp", bufs=1)
            nc.tensor.matmul(out=erp[:, 0:n], lhsT=ones1[:],
                             rhs=e1row[0:1, k0:k0 + n], start=True, stop=True)
            nc.vector.tensor_copy(out=e1rep[:, k0:k0 + n], in_=erp[:, 0:n])
        if E1G:
            # e1tab[g*128+p, 0:F] = e1 (row-replicated) in DRAM for the
            # gather-based e1 path
            e1sb = cpool.tile([P, Gc * F], f32)
            nc.vector.tensor_scalar(
                out=e1sb[:].rearrange("p (g f) -> p g f", f=F),
                in0=e1_all[:, :, None].to_broadcast([P, Gc, F]),
                scalar1=0.0, scalar2=None, op0=OP.add)
            e1tab = ins["e1tab"]
            nc.sync.dma_start(
                out=e1tab.rearrange("(g p) f -> p g f", p=P),
                in_=e1sb[:].rearrange("p (g f) -> p g f", f=F))
            tc.strict_bb_all_engine_barrier()

        # ---------------- per-batch compute ---------------------------
        batch_tiles = {}
        gather_ctr = [0]

        def get_batch(X, b):
            key = (X, b)
            if key in batch_tiles:
                return batch_tiles[key]
            gi = pool.tile([P, CB * P // 16], i16, tag=f"gi{X}")
            nc.sync.dma_start(out=gi[:], in_=gidx_all[bofs[X] + b])
            sl = pool.tile([P, CB], f32, tag=f"sl{X}")
            nc.sync.dma_start(out=sl[:], in_=sloc_all[bofs[X] + b])
            rows = pool.tile([P, CB * F], f32, tag=f"rows{X}", bufs=4)
            rows3 = rows[:].rearrange("p (c f) -> p c f", f=F)
            if "nogather" not in abl:
                for h0 in range(0, CB, 8):
                    nc.gpsimd.dma_gather(
                        out_ap=rows3[:, h0:h0 + 8, :], in_ap=halves[X],
                        idxs_ap=gi[:, h0 * 8:(h0 + 8) * 8],
                        num_idxs=8 * P, num_idxs_reg=8 * P, elem_size=F,
                        queue_num=gather_ctr[0] % NQ)
                    gather_ctr[0] += 1
            else:
                gather_ctr[0] += 2

            # q[p, c] = <rows[p, c, :], Wa2>
            qprod = pool.tile([P, CB * F], f32, tag=f"qp{X}")
            nc.vector.tensor_tensor(
                out=qprod[:].rearrange("p (c f) -> p c f", f=F),
                in0=rows3,
                in1=w2v[:, None, :].to_broadcast([P, CB, F]), op=OP.mult)
            q_t = pool.tile([P, CB], f32, tag=f"q{X}")
            nc.vector.tensor_reduce(
                out=q_t[:],
                in_=qprod[:].rearrange("p (c f) -> p c f", f=F),
                axis=AX.X, op=OP.add)

            # onehot[e, c, n] = (sloc[e, c] == n)
            oh = pool.tile([P, CB * P], bf16, tag=f"oh{X}")
            oh3 = oh[:].rearrange("p (c n) -> p c n", n=P)
            nc.vector.tensor_tensor(
                out=oh3,
                in0=sl[:, :, None].to_broadcast([P, CB, P]),
                in1=iota_t.rearrange("p (c n) -> p c n", n=P),
                op=OP.is_equal)

            # e1 per edge, two load-balanced mechanisms: every 4th batch
            # gathers e1 rows from e1tab (DMA path), the rest select via
            # onehot x e1rep reduce on DVE
            use_g = (E1G > 0 and gather_ctr[0] % E1G == 0
                     and "noe1" not in abl)
            if use_g:
                gi2 = pool.tile([P, CB * P // 16], i16, tag=f"gi2{X}")
                nc.sync.dma_start(out=gi2[:], in_=gidx2[X][b])
                e1g = pool.tile([P, CB * F], f32, tag=f"e1g{X}", bufs=4)
                e1g3 = e1g[:].rearrange("p (c f) -> p c f", f=F)
                nc.gpsimd.dma_gather(
                    out_ap=e1g3, in_ap=e1tab, idxs_ap=gi2[:],
                    num_idxs=CB * P, num_idxs_reg=CB * P, elem_size=F,
                    queue_num=gather_ctr[0] % NQ)
                gather_ctr[0] += 1
                ep_src = e1g3[:, :, 0]
            else:
                bm = meta[X]["batches"][b]
                ep_all = pool.tile([P, CB], f32, tag=f"ep{X}")
                eprod = pool.tile([P, CB * P], bf16, tag=f"eprod{X}")
                for (c0, c1, g) in (bm["runs"] if "noe1" not in abl else []):
                    nc.vector.tensor_tensor(
                        out=eprod[:].rearrange("p (c n) -> p c n", n=P)
                            [:, c0:c1, :],
                        in0=oh3[:, c0:c1, :],
                        in1=e1rep[:, g * P:(g + 1) * P][:, None, :]
                            .to_broadcast([P, c1 - c0, P]),
                        op=OP.mult)
                    nc.vector.tensor_reduce(
                        out=ep_all[:, c0:c1],
                        in_=eprod[:].rearrange("p (c n) -> p c n", n=P)
                            [:, c0:c1, :],
                        axis=AX.X, op=OP.add)
                if "noe1" in abl:
                    nc.vector.memset(ep_all[:], 0.0)
                ep_src = ep_all[:]

            # score = exp(elu(q + e1, 0.1))
            s_t = pool.tile([P, CB], f32, tag=f"s{X}")
            nc.vector.tensor_tensor(out=s_t[:], in0=q_t[:],
                                    in1=ep_src, op=OP.add)
            t_t = pool.tile([P, CB], f32, tag=f"t{X}")
            nc.scalar.activation(t_t[:], s_t[:], ACTF.Exp,
                                 bias=zbias[:, 0:1], scale=1.0)
            u_t = pool.tile([P, CB], f32, tag=f"u{X}")
            nc.scalar.activation(u_t[:], t_t[:], ACTF.Exp,
                                 bias=mbias[:, 0:1], scale=0.1)
            m_t = pool.tile([P, CB], u8, tag=f"m{X}")
            nc.vector.tensor_scalar(
                out=m_t[:], in0=s_t[:], scalar1=0.0, scalar2=None,
                op0=OP.is_gt)
            nc.vector.copy_predicated(out=u_t[:], mask=m_t[:], data=t_t[:])

            # rhs65[e, c, :] = [score * fb_row | score]  (bf16)
            rhs = pool.tile([P, CB * (F + 1)], bf16, tag=f"rhs{X}")
            rhs3 = rhs[:].rearrange("p (c w) -> p c w", w=F + 1)
            nc.vector.tensor_tensor(
                out=rhs3[:, :, 0:F], in0=rows3,
                in1=u_t[:, :, None].to_broadcast([P, CB, F]), op=OP.mult)
            nc.vector.tensor_copy(out=rhs3[:, :, F], in_=u_t[:])

            res = (oh3, rhs3)
            batch_tiles[key] = res
            # keep the dict small: drop entries older than a few batches
            for k in list(batch_tiles):
                if k[0] == X and k[1] < b - 2:
                    del batch_tiles[k]
            return res

        # ---------------- group-major matmul + epilogue ---------------
        out3 = out.rearrange("(g p) f -> g p f", p=P)
        outs_sb = None
        for g in range(Gc):
            acc = psum.tile([P, F + 1], f32, tag="acc")
            first = True
            for X in (0, 1):
                for (b, c) in meta[X]["group_chunks"][g]:
                    oh3, rhs3 = get_batch(X, b)
                    last = (X == 1) and (b, c) == meta[1]["group_chunks"][g][-1]
                    if "nomm" not in abl:
                        nc.tensor.matmul(
                            out=acc[:], lhsT=oh3[:, c, :], rhs=rhs3[:, c, :],
                            start=first, stop=last)
                    first = False

            # epilogue for group g
            if "noep" in abl or "nomm" in abl:
                continue
            den = acc[:, F:F + 1]
            m0 = pool.tile([P, 1], f32, tag="m0")
            nc.vector.tensor_scalar(out=m0[:], in0=den, scalar1=0.0,
                                    scalar2=None, op0=OP.is_equal)
            w1 = pool.tile([P, 1], f32, tag="w1")
            nc.vector.tensor_scalar(out=w1[:], in0=den, scalar1=0.0,
                                    scalar2=None, op0=OP.is_gt)
            dsafe = pool.tile([P, 1], f32, tag="dsafe")
            nc.vector.tensor_tensor(out=dsafe[:], in0=den, in1=m0[:],
                                    op=OP.add)
            rec = pool.tile([P, 1], f32, tag="rec")
            nc.vector.reciprocal(rec[:], dsafe[:])
            h_t = pool.tile([P, F], f32, tag="h")
            nc.vector.tensor_scalar(out=h_t[:], in0=acc[:, 0:F],
                                    scalar1=rec[:, 0:1], scalar2=None,
                                    op0=OP.mult)
            htp = psep.tile([F, P], f32, tag="htp")
            nc.tensor.transpose(out=htp[:], in_=h_t[:], identity=ident[:])
            ht = pool.tile([F, P], f32, tag="ht")
            nc.vector.tensor_copy(out=ht[:], in_=htp[:])
            op_t = psep.tile([P, F], f32, tag="op")
            nc.tensor.matmul(out=op_t[:], lhsT=ht[:], rhs=wmat_t[0:F, :],
                             start=True, stop=True)
            badd = pool.tile([P, F], f32, tag="badd")
            nc.vector.tensor_scalar(out=badd[:], in0=bv,
                                    scalar1=w1[:, 0:1], scalar2=None,
                                    op0=OP.mult)
            if g % 7 == 0:
                outs_sb = pool.tile([P, 7 * F], f32, tag="outs")
            nc.vector.tensor_tensor(
                out=outs_sb[:, (g % 7) * F:(g % 7 + 1) * F],
                in0=op_t[:], in1=badd[:], op=OP.add)
            if g % 7 == 6:
                g0 = g - 6
                nc.sync.dma_start(
                    out=out3[g0:g0 + 7].rearrange("g p f -> p g f"),
                    in_=outs_sb[:].rearrange("p (g f) -> p g f", f=F))


# ----------------------------------------------------------------------------
# host-side preparation (index plumbing only)
# ----------------------------------------------------------------------------

def prep_inputs(feature_a, feature_b, W, b, a_vec, edges, node_num_a,
                ncores=NCORES):
    fa = np.asarray(feature_a, np.float32)
    fb = np.asarray(feature_b, np.float32)
    W = np.asarray(W, np.float32)
    b = np.asarray(b, np.float32)
    a_vec = np.asarray(a_vec, np.float32).reshape(-1)
    edges = np.asarray(edges)
    NA = int(node_num_a)
    NB, Fdim = fb.shape
    assert Fdim == F and fa.shape[1] == F

    a1 = a_vec[:F]
    a2 = a_vec[F:]
    Wa2 = (W @ a2).astype(np.float32)
    ba2 = float(b @ a2)

    nodes_per_core = -(-NA // (ncores * P)) * P          # 6272
    Gc = nodes_per_core // P                             # 49
    NA_pad = nodes_per_core * ncores

    H = -(-NB // 256) * 128                              # 25088 (<= 32768)
    assert H <= 32768 and NB <= 2 * H
    fb_pad = np.zeros((2 * H, F), np.float32)
    fb_pad[:NB] = fb

    src = edges[:, 0].astype(np.int64)
    dst = edges[:, 1].astype(np.int64)
    core = src // nodes_per_core
    half = (dst >= H).astype(np.int64)
    dloc = dst - half * H
    sl_node = src - core * nodes_per_core
    g_all = sl_node // P
    sloc_all = sl_node % P

    # per (core, half, group) counts -> cross-core uniform chunk counts
    cell = (core * 2 + half) * Gc + g_all
    counts = np.bincount(cell, minlength=ncores * 2 * Gc) \
        .reshape(ncores, 2, Gc)
    nch = np.maximum(1, -(-counts.max(axis=0) // P))     # [2, Gc]

    # chunk id layout per half: group-major chunk streams
    chunk_of_g = [np.concatenate([[0], np.cumsum(nch[X])]) for X in (0, 1)]
    nch_tot = [int(nch[X].sum()) for X in (0, 1)]
    NBATCH = [-(-nch_tot[X] // CB) for X in (0, 1)]

    # per-half metadata (identical for all cores)
    meta = []
    for X in (0, 1):
        chunk_groups = np.full(NBATCH[X] * CB, -1, np.int64)
        chunk_groups[:nch_tot[X]] = np.repeat(np.arange(Gc), nch[X])
        batches = []
        for bi in range(NBATCH[X]):
            cg = chunk_groups[bi * CB:(bi + 1) * CB]
            runs = []
            cprev = 0
            for c in range(1, CB + 1):
                if c == CB or cg[c] != cg[cprev]:
                    if cg[cprev] >= 0:
                        runs.append((cprev, c, int(cg[cprev])))
                    cprev = c
            batches.append(dict(runs=runs))
        group_chunks = []
        for g in range(Gc):
            lo, hi = chunk_of_g[X][g], chunk_of_g[X][g + 1]
            group_chunks.append([(int(ci // CB), int(ci % CB))
                                 for ci in range(lo, hi)])
        meta.append(dict(batches=batches, group_chunks=group_chunks))

    iota8 = np.tile(np.arange(P, dtype=np.float32)[None, :], (P, CB))

    wvec = np.zeros((P, 3 * F), np.float32)
    wvec[:, 0:F] = a1[None, :]
    wvec[:, F:2 * F] = Wa2[None, :]
    wvec[:, 2 * F:3 * F] = b[None, :]

    in_maps = []
    for c in range(ncores):
        msk = core == c
        ehalf = half[msk]
        edloc = dloc[msk]
        eg = g_all[msk]
        esloc = sloc_all[msk]
        order = np.lexsort((esloc, eg, ehalf))
        ehalf, edloc, eg, esloc = (x[order] for x in
                                   (ehalf, edloc, eg, esloc))
        gidx_maps = []
        gidx2_maps = []
        sloc_maps = []
        for X in (0, 1):
            nslot = NBATCH[X] * CB * P
            dsl = np.zeros(nslot, np.int64)
            nsl = np.zeros(nslot, np.int64)
            ssl = np.full(nslot, -1.0, np.float32)
            selX = ehalf == X
            dX = edloc[selX]
            gX = eg[selX]
            sX = esloc[selX]
            cnts = np.bincount(gX, minlength=Gc)
            off = np.concatenate([[0], np.cumsum(cnts)])
            for g in range(Gc):
                n = int(cnts[g])
                base = int(chunk_of_g[X][g]) * P
                dsl[base:base + n] = dX[off[g]:off[g] + n]
                nsl[base:base + n] = g * P + sX[off[g]:off[g] + n]
                ssl[base:base + n] = sX[off[g]:off[g] + n]
            # pack gather indices: flat[i = c*128+e]; 16-wrap + 8x tile
            dslb = dsl.reshape(NBATCH[X], CB * P)
            nslb = nsl.reshape(NBATCH[X], CB * P)
            gmaps = np.zeros((NBATCH[X], P, CB * P // 16), np.int16)
            g2maps = np.zeros((NBATCH[X], P, CB * P // 16), np.int16)
            smaps = np.zeros((NBATCH[X], P, CB), np.float32)
            for bi in range(NBATCH[X]):
                sb = dslb[bi].reshape(CB * P // 16, 16).T.astype(np.int16)
                gmaps[bi] = np.tile(sb, (8, 1))
                sb2 = nslb[bi].reshape(CB * P // 16, 16).T.astype(np.int16)
                g2maps[bi] = np.tile(sb2, (8, 1))
                smaps[bi] = ssl[bi * CB * P:(bi + 1) * CB * P] \
                    .reshape(CB, P).T
            gidx_maps.append(gmaps)
            gidx2_maps.append(g2maps)
            sloc_maps.append(smaps)

        fa_core = np.zeros((nodes_per_core, F), np.float32)
        lo = c * nodes_per_core
        hi = min(lo + nodes_per_core, NA)
        fa_core[:hi - lo] = fa[lo:hi]
        fa_pk = np.ascontiguousarray(
            fa_core.reshape(Gc, P, F).transpose(1, 0, 2).reshape(P, Gc * F))

        wmat_pad = np.zeros((P, F), np.float32)
        wmat_pad[0:F] = W
        consts = np.ascontiguousarray(
            np.concatenate([iota8, wvec, wmat_pad, fa_pk], axis=1))
        in_maps.append(dict(
            fb_pad=fb_pad,
            consts=consts,
            gidx_all=np.ascontiguousarray(
                np.concatenate([gidx_maps[0], gidx_maps[1]], axis=0)),
            sloc_all=np.ascontiguousarray(
                np.concatenate([sloc_maps[0], sloc_maps[1]], axis=0)),
            **(dict(gidx2A=gidx2_maps[0], gidx2B=gidx2_maps[1])
               if E1G else {}),
        ))

    cfg = dict(Gc=Gc, H=H, ba2=ba2, meta=meta, NBATCH=NBATCH,
               nodes_per_core=nodes_per_core, NA=NA)
    return in_maps, cfg


def build_bass(cfg, ncores=NCORES):
    nc = bacc.Bacc("TRN2", target_bir_lowering=False, debug=False,
                   enable_asserts=False, num_devices=ncores,
                   num_swdge_queues=NQ)
    Gc, H = cfg["Gc"], cfg["H"]
    NBATCH = cfg["NBATCH"]
    NBT = NBATCH[0] + NBATCH[1]
    CW = CB * P + 3 * F + F + Gc * F
    ins = dict(
        fb_pad=nc.dram_tensor("fb_pad", [2 * H, F], f32,
                              kind="ExternalInput").ap(),
        consts=nc.dram_tensor("consts", [P, CW], f32,
                              kind="ExternalInput").ap(),
        gidx_all=nc.dram_tensor("gidx_all", [NBT, P, CB * P // 16], i16,
                                kind="ExternalInput").ap(),
        sloc_all=nc.dram_tensor("sloc_all", [NBT, P, CB], f32,
                                kind="ExternalInput").ap(),

        e1d=nc.dram_tensor("e1d", [Gc, P], f32, kind="Internal").ap(),

    )
    outs = dict(
        out=nc.dram_tensor("out", [Gc * P, F], f32,
                           kind="ExternalOutput").ap(),
    )
    with tile.TileContext(nc) as tc:
        emit_program(tc, ins, outs, cfg)
    nc.compile()
    return nc


# ----------------------------------------------------------------------------
# entry point
# ----------------------------------------------------------------------------

def kernel(**inputs):
    from concourse import bass_utils

    in_maps, cfg = prep_inputs(**inputs)
    nc = build_bass(cfg)
    res = bass_utils.run_bass_kernel_spmd(
        nc, in_maps, core_ids=list(range(NCORES)))
    outs = [r["out"][:cfg["nodes_per_core"]] for r in res.results]
    full = np.concatenate(outs, axis=0)[:cfg["NA"]]
    return full.astype(np.float32)


# ----------------------------------------------------------------------------
# numpy emulation of the device program (for host-prep validation)
# ----------------------------------------------------------------------------

def emulate(in_maps, cfg):
    Gc, H = cfg["Gc"], cfg["H"]
    meta = cfg["meta"]
    ba2 = cfg["ba2"]
    outs = []
    o_wvec = CB * P
    o_wmat, o_fa = CB * P + 3 * F, CB * P + 3 * F + F
    for im in in_maps:
        fb_pad = im["fb_pad"]
        consts = im["consts"]
        fa = consts[:, o_fa:o_fa + Gc * F].reshape(P, Gc, F) \
            .transpose(1, 0, 2).reshape(Gc * P, F)
        a1 = consts[0, o_wvec:o_wvec + F]
        Wa2 = consts[0, o_wvec + F:o_wvec + 2 * F]
        bvec = consts[0, o_wvec + 2 * F:o_wvec + 3 * F]
        W = consts[0:F, o_wmat:o_wmat + F]
        e1 = fa @ a1 + ba2                       # [Gc*P]
        acc = np.zeros((Gc * P, F + 1), np.float64)
        nb0 = cfg["NBATCH"][0]
        for X, ofs in ((0, 0), (1, nb0)):
            NBx = cfg["NBATCH"][X]
            gm = im["gidx_all"][ofs:ofs + NBx]
            sm = im["sloc_all"][ofs:ofs + NBx]
            for bi in range(NBx):
                # unpack idx: [128, CB*P//16] -> flat
                sb = gm[bi][:16]                 # [16, CB*P//16]
                flat = sb.T.reshape(-1).astype(np.int64)   # i = c*128+e
                didx = flat.reshape(CB, P)       # [c, e]
                sl = sm[bi]                      # [e, c]
                rows = fb_pad[X * H + didx]      # [c, e, F]
                q = rows @ Wa2                   # [c, e]
                for (c0, c1, g) in meta[X]["batches"][bi]["runs"]:
                    for c in range(c0, c1):
                        sle = sl[:, c]
                        valid = sle >= 0
                        s = q[c] + e1[g * P + (sle.astype(np.int64) % P)]
                        score = np.where(
                            s > 0, np.exp(s),
                            np.exp(0.1 * np.exp(np.minimum(s, 0)) - 0.1))
                        score = score * valid
                        onehot = (sle[:, None] ==
                                  np.arange(P)[None, :])   # [e, n]
                        acc[g * P:(g + 1) * P, 0:F] += \
                            onehot.T @ (score[:, None] * rows[c])
                        acc[g * P:(g + 1) * P, F] += onehot.T @ score
        den = acc[:, F]
        dsafe = np.where(den == 0, 1.0, den)
        h = acc[:, 0:F] / dsafe[:, None]
        o = h @ W + (den > 0)[:, None] * bvec[None, :]
        outs.append(o.astype(np.float32))
    full = np.concatenate(outs, 0)[:cfg["NA"]]
    return full


if __name__ == "__main__":
    np.random.seed(0)
    NA = NB = 50000
    E = 800000
    ins = dict(
        feature_a=np.random.randn(NA, F).astype(np.float32),
        feature_b=np.random.randn(NB, F).astype(np.float32),
        W=(np.random.randn(F, F) / 8).astype(np.float32),
        b=(np.random.randn(F) * 0.1).astype(np.float32),
        a_vec=(np.random.randn(2 * F, 1) * 0.05).astype(np.float32),
        edges=np.stack([np.random.randint(0, NA, E),
                        np.random.randint(0, NB, E)], 1).astype(np.int64),
        node_num_a=NA,
    )
    # numpy reference
    fa, fb = ins["feature_a"], ins["feature_b"]
    W, b_, av = ins["W"], ins["b"], ins["a_vec"].reshape(-1)
    src, dst = ins["edges"][:, 0], ins["edges"][:, 1]
    new_emb = fb @ W + b_
    s = (fa @ av[:F])[src] + (new_emb @ av[F:])[dst]
    score = np.exp(np.where(s > 0, s, 0.1 * (np.exp(np.minimum(s, 0)) - 1)))
    num = np.zeros((NA, F))
    np.add.at(num, src, score[:, None] * new_emb[dst])
    den = np.zeros(NA)
    np.add.at(den, src, score)
    dsafe = np.where(den == 0, 1, den)
    expected = num / dsafe[:, None]

    in_maps, cfg = prep_inputs(**ins)
    print("NBATCH:", cfg["NBATCH"], "slots:",
          sum(n * CB * P for n in cfg["NBATCH"]))
    got = emulate(in_maps, cfg)
    rel = np.linalg.norm(got - expected) / np.linalg.norm(expected)
    print("emulation rel err:", rel)


# revision 6
# speedup vs baseline: 9507.0676x; 1.3190x over previous
"""Trainium2 Bass kernel for AttentionAggregator (GNN message passing), v2.

Reference computation:
    new_emb = fb @ W + b
    s_e     = (fa @ a1)[src_e] + (new_emb @ a2)[dst_e]
    score_e = exp(elu(s_e, 0.1))
    out[n]  = (sum_{e: src_e=n} score_e * new_emb[dst_e]) / max(den[n], 1 if 0)

Reformulation (linearity of the segment sum):
    q_e   = fb[dst_e] @ (W @ a2)
    s_e   = (fa @ a1 + b @ a2)[src_e] + q_e
    G[n]  = sum_e score_e * fb[dst_e];  den[n] = sum_e score_e
    out[n]= (G[n] / den_safe[n]) @ W + 1[den>0] * b

Distribution: src nodes sharded contiguously across 8 cores (6272 rows
each); fb replicated.  No cross-core collective needed.

Device algorithm (per core):
  - Edges sorted by (dst-half, src).  Per src-group g (128 nodes) and
    dst-half X, edges are padded into chunks of 128 slots.  Chunk counts
    per (g, X) are cross-core uniform (max over cores) so one SPMD
    program serves all cores.
  - Chunks stream in batches of 8 (=1024 slots): one dma_gather per
    batch fetches fb[dst] f32 rows (256B) straight from the fb input,
    rotating over 4 SWDGE queues (~1.4 ns/row).
  - Per batch (vector/scalar engines): q = <row, W@a2>; one-hot
    onehot[e,n] = (srcloc[e]==n) via iota compare; e1 per edge via
    onehot x replicated-e1 reduce; score = exp(elu(q+e1, 0.1)); rhs65 =
    [score*fb | score] in bf16.
  - Segment sum on the PE: per chunk, matmul(onehot^T @ rhs65)
    accumulates [128 nodes x 65] (G | den) in a PSUM bank; group-major
    consumption keeps only ~3 accumulator banks live.
  - Epilogue per group: divide by den_safe, transpose via PE, multiply
    by W, add (den>0)*b, DMA out.
"""

import sys

for _p in ("/opt/trn_rl_repo",):
    if _p not in sys.path:
        sys.path.insert(0, _p)

import numpy as np

import concourse.bass as bass
import concourse.bacc as bacc
import concourse.mybir as mybir
import concourse.tile as tile
from concourse.masks import make_identity

P = 128
F = 64
NCORES = 8
CB = 16           # chunks per batch (two 1024-idx dma_gathers per batch)
NQ = 4            # SWDGE queues
E1G = 0           # if >0, every E1G-th batch uses the e1 dma_gather path
                  # (measured slower than pure DVE select on this part)

f32 = mybir.dt.float32
bf16 = mybir.dt.bfloat16
i16 = mybir.dt.int16
u8 = mybir.dt.uint8
AX = mybir.AxisListType
OP = mybir.AluOpType
ACTF = mybir.ActivationFunctionType


# ----------------------------------------------------------------------------
# device program
# ----------------------------------------------------------------------------

def emit_program(tc, ins, outs, cfg):
    nc = tc.nc
    abl = cfg.get("ablate", set())
    Gc = cfg["Gc"]                      # 49 src groups
    H = cfg["H"]                        # rows per dst half
    meta = cfg["meta"]                  # per-half batch/chunk metadata
    ba2 = float(cfg["ba2"])
    NBATCH = cfg["NBATCH"]
    fb_pad = ins["fb_pad"]              # [2H, F] f32
    consts = ins["consts"]              # [P, CW]: iota8 | wvec | wmat | fa_pk
    gidx_all = ins["gidx_all"]          # [NB0+NB1, P, CB*P//16] i16
    sloc_all = ins["sloc_all"]          # [NB0+NB1, P, CB] f32
    bofs = {0: 0, 1: NBATCH[0]}
    gidx2 = ({0: ins["gidx2A"], 1: ins["gidx2B"]} if E1G else None)
    out = outs["out"]                   # [Gc*P, F] f32

    halves = {0: fb_pad[0:H, :], 1: fb_pad[H:2 * H, :]}

    with (
        tc.tile_pool(name="const", bufs=1) as cpool,
        tc.tile_pool(name="work", bufs=3) as pool,
        tc.tile_pool(name="psum", bufs=3, space="PSUM") as psum,
        tc.tile_pool(name="psep", bufs=2, space="PSUM") as psep,
    ):
        # ---------------- constants (one block load) -------------------
        CW = CB * P + 3 * F + F + Gc * F
        cbig = cpool.tile([P, CW], f32)
        nc.sync.dma_start(out=cbig[:], in_=consts)
        o_iota, o_wvec = 0, CB * P
        o_wmat, o_fa = CB * P + 3 * F, CB * P + 3 * F + F
        iota_t = cbig[:, o_iota:o_iota + CB * P]
        a1v = cbig[:, o_wvec:o_wvec + F]
        w2v = cbig[:, o_wvec + F:o_wvec + 2 * F]
        bv = cbig[:, o_wvec + 2 * F:o_wvec + 3 * F]
        wmat_t = cbig[:, o_wmat:o_wmat + F]
        fa_t = cbig[:, o_fa:o_fa + Gc * F]
        ident = cpool.tile([P, P], f32)
        make_identity(nc, ident[:])
        zbias = cpool.tile([P, 1], f32)
        nc.vector.memset(zbias[:], 0.0)
        mbias = cpool.tile([P, 1], f32)
        nc.vector.memset(mbias[:], -0.1)

        # ---------------- e1 = fa @ a1 + ba2, replicated --------------
        fprod = cpool.tile([P, Gc * F], f32)
        nc.vector.tensor_tensor(
            out=fprod[:].rearrange("p (g f) -> p g f", f=F),
            in0=fa_t.rearrange("p (g f) -> p g f", f=F),
            in1=a1v[:, None, :].to_broadcast([P, Gc, F]), op=OP.mult)
        e1_all = cpool.tile([P, Gc], f32)
        nc.vector.tensor_reduce(
            out=e1_all[:],
            in_=fprod[:].rearrange("p (g f) -> p g f", f=F),
            axis=AX.X, op=OP.add)
        nc.vector.tensor_scalar(
            out=e1_all[:], in0=e1_all[:], scalar1=ba2, scalar2=None,
            op0=OP.add)
        # replicate e1 across partitions: e1rep[p, g*128+n] = e1[g*128+n].
        # Route through DRAM to flatten [128, Gc] -> one [1, Gc*128] row
        # (PE rhs base partition must be 0), then ones-matmul broadcast.
        ones1 = cpool.tile([1, P], f32)
        nc.vector.memset(ones1[:], 1.0)
        e1tp = psep.tile([Gc, P], f32, tag="htp")
        nc.tensor.transpose(out=e1tp[:], in_=e1_all[:], identity=ident[:])
        e1tps = cpool.tile([Gc, P], f32)
        nc.vector.tensor_copy(out=e1tps[:], in_=e1tp[:])
        e1d = ins["e1d"]
        nc.sync.dma_start(out=e1d, in_=e1tps[:])
        e1row = cpool.tile([1, Gc * P], f32)
        nc.sync.dma_start(out=e1row[0:1, :],
                          in_=e1d.rearrange("g p -> (g p)")[None, :])
        e1rep = cpool.tile([P, Gc * P], bf16)
        for k0 in range(0, Gc * P, 512):
            n = min(512, Gc * P - k0)
            erp = psep.tile([P, 512], f32, tag="erp", bufs=1)
            nc.tensor.matmul(out=erp[:, 0:n], lhsT=ones1[:],
                             rhs=e1row[0:1, k0:k0 + n], start=True, stop=True)
            nc.vector.tensor_copy(out=e1rep[:, k0:k0 + n], in_=erp[:, 0:n])
        if E1G:
            # e1tab[g*128+p, 0:F] = e1 (row-replicated) in DRAM for the
            # gather-based e1 path
            e1sb = cpool.tile([P, Gc * F], f32)
            nc.vector.tensor_scalar(
                out=e1sb[:].rearrange("p (g f) -> p g f", f=F),
                in0=e1_all[:, :, None].to_broadcast([P, Gc, F]),
                scalar1=0.0, scalar2=None, op0=OP.add)
            e1tab = ins["e1tab"]
            nc.sync.dma_start(
                out=e1tab.rearrange("(g p) f -> p g f", p=P),
                in_=e1sb[:].rearrange("p (g f) -> p g f", f=F))
            tc.strict_bb_all_engine_barrier()

        # ---------------- per-batch compute ---------------------------
        batch_tiles = {}
        gather_ctr = [0]

        def get_batch(X, b):
            key = (X, b)
            if key in batch_tiles:
                return batch_tiles[key]
            gi = pool.tile([P, CB * P // 16], i16, tag=f"gi{X}")
            nc.sync.dma_start(out=gi[:], in_=gidx_all[bofs[X] + b])
            sl = pool.tile([P, CB], f32, tag=f"sl{X}")
            nc.sync.dma_start(out=sl[:], in_=sloc_all[bofs[X] + b])
            rows = pool.tile([P, CB * F], f32, tag=f"rows{X}", bufs=4)
            rows3 = rows[:].rearrange("p (c f) -> p c f", f=F)
            if "nogather" not in abl:
                for h0 in range(0, CB, 8):
                    nc.gpsimd.dma_gather(
                        out_ap=rows3[:, h0:h0 + 8, :], in_ap=halves[X],
                        idxs_ap=gi[:, h0 * 8:(h0 + 8) * 8],
                        num_idxs=8 * P, num_idxs_reg=8 * P, elem_size=F,
                        queue_num=gather_ctr[0] % NQ)
                    gather_ctr[0] += 1
            else:
                gather_ctr[0] += 2

            # q[p, c] = <rows[p, c, :], Wa2>
            qprod = pool.tile([P, CB * F], f32, tag=f"qp{X}")
            nc.vector.tensor_tensor(
                out=qprod[:].rearrange("p (c f) -> p c f", f=F),
                in0=rows3,
                in1=w2v[:, None, :].to_broadcast([P, CB, F]), op=OP.mult)
            q_t = pool.tile([P, CB], f32, tag=f"q{X}")
            nc.vector.tensor_reduce(
                out=q_t[:],
                in_=qprod[:].rearrange("p (c f) -> p c f", f=F),
                axis=AX.X, op=OP.add)

            # onehot[e, c, n] = (sloc[e, c] == n)
            oh = pool.tile([P, CB * P], bf16, tag=f"oh{X}")
            oh3 = oh[:].rearrange("p (c n) -> p c n", n=P)
            nc.vector.tensor_tensor(
                out=oh3,
                in0=sl[:, :, None].to_broadcast([P, CB, P]),
                in1=iota_t.rearrange("p (c n) -> p c n", n=P),
                op=OP.is_equal)

            # e1 per edge, two load-balanced mechanisms: every 4th batch
            # gathers e1 rows from e1tab (DMA path), the rest select via
            # onehot x e1rep reduce on DVE
            use_g = (E1G > 0 and gather_ctr[0] % E1G == 0
                     and "noe1" not in abl)
            if use_g:
                gi2 = pool.tile([P, CB * P // 16], i16, tag=f"gi2{X}")
                nc.sync.dma_start(out=gi2[:], in_=gidx2[X][b])
                e1g = pool.tile([P, CB * F], f32, tag=f"e1g{X}", bufs=4)
                e1g3 = e1g[:].rearrange("p (c f) -> p c f", f=F)
                nc.gpsimd.dma_gather(
                    out_ap=e1g3, in_ap=e1tab, idxs_ap=gi2[:],
                    num_idxs=CB * P, num_idxs_reg=CB * P, elem_size=F,
                    queue_num=gather_ctr[0] % NQ)
                gather_ctr[0] += 1
                ep_src = e1g3[:, :, 0]
            else:
                bm = meta[X]["batches"][b]
                ep_all = pool.tile([P, CB], f32, tag=f"ep{X}")
                eprod = pool.tile([P, CB * P], bf16, tag=f"eprod{X}")
                for (c0, c1, g) in (bm["runs"] if "noe1" not in abl else []):
                    nc.vector.tensor_tensor(
                        out=eprod[:].rearrange("p (c n) -> p c n", n=P)
                            [:, c0:c1, :],
                        in0=oh3[:, c0:c1, :],
                        in1=e1rep[:, g * P:(g + 1) * P][:, None, :]
                            .to_broadcast([P, c1 - c0, P]),
                        op=OP.mult)
                    nc.vector.tensor_reduce(
                        out=ep_all[:, c0:c1],
                        in_=eprod[:].rearrange("p (c n) -> p c n", n=P)
                            [:, c0:c1, :],
                        axis=AX.X, op=OP.add)
                if "noe1" in abl:
                    nc.vector.memset(ep_all[:], 0.0)
                ep_src = ep_all[:]

            # score = exp(elu(q + e1, 0.1))
            s_t = pool.tile([P, CB], f32, tag=f"s{X}")
            nc.vector.tensor_tensor(out=s_t[:], in0=q_t[:],
                                    in1=ep_src, op=OP.add)
            t_t = pool.tile([P, CB], f32, tag=f"t{X}")
            nc.scalar.activation(t_t[:], s_t[:], ACTF.Exp,
                                 bias=zbias[:, 0:1], scale=1.0)
            u_t = pool.tile([P, CB], f32, tag=f"u{X}")
            nc.scalar.activation(u_t[:], t_t[:], ACTF.Exp,
                                 bias=mbias[:, 0:1], scale=0.1)
            m_t = pool.tile([P, CB], u8, tag=f"m{X}")
            nc.vector.tensor_scalar(
                out=m_t[:], in0=s_t[:], scalar1=0.0, scalar2=None,
                op0=OP.is_gt)
            nc.vector.copy_predicated(out=u_t[:], mask=m_t[:], data=t_t[:])

            # rhs65[e, c, :] = [score * fb_row | score]  (bf16)
            rhs = pool.tile([P, CB * (F + 1)], bf16, tag=f"rhs{X}")
            rhs3 = rhs[:].rearrange("p (c w) -> p c w", w=F + 1)
            nc.vector.tensor_tensor(
                out=rhs3[:, :, 0:F], in0=rows3,
                in1=u_t[:, :, None].to_broadcast([P, CB, F]), op=OP.mult)
            nc.vector.tensor_copy(out=rhs3[:, :, F], in_=u_t[:])

            res = (oh3, rhs3)
            batch_tiles[key] = res
            # keep the dict small: drop entries older than a few batches
            for k in list(batch_tiles):
                if k[0] == X and k[1] < b - 2:
                    del batch_tiles[k]
            return res

        # ---------------- group-major matmul + epilogue ---------------
        out3 = out.rearrange("(g p) f -> g p f", p=P)
        outs_sb = None
        for g in range(Gc):
            acc = psum.tile([P, F + 1], f32, tag="acc")
            first = True
            for X in (0, 1):
                for (b, c) in meta[X]["group_chunks"][g]:
                    oh3, rhs3 = get_batch(X, b)
                    last = (X == 1) and (b, c) == meta[1]["group_chunks"][g][-1]
                    if "nomm" not in abl:
                        nc.tensor.matmul(
                            out=acc[:], lhsT=oh3[:, c, :], rhs=rhs3[:, c, :],
                            start=first, stop=last)
                    first = False

            # epilogue for group g
            if "noep" in abl or "nomm" in abl:
                continue
            den = acc[:, F:F + 1]
            bzero = cfg.get("bzero", False)
            dsafe = pool.tile([P, 1], f32, tag="dsafe")
            if bzero:
                # b == 0: dsafe = max(den, tiny) makes den==0 rows (G==0)
                # come out exactly 0, and all bias machinery drops out
                nc.vector.tensor_scalar(out=dsafe[:], in0=den,
                                        scalar1=1e-30, scalar2=None,
                                        op0=OP.max)
            else:
                m0 = pool.tile([P, 1], f32, tag="m0")
                nc.vector.tensor_scalar(out=m0[:], in0=den, scalar1=0.0,
                                        scalar2=None, op0=OP.is_equal)
                w1 = pool.tile([P, 1], f32, tag="w1")
                nc.vector.tensor_scalar(out=w1[:], in0=den, scalar1=0.0,
                                        scalar2=None, op0=OP.is_gt)
                nc.vector.tensor_tensor(out=dsafe[:], in0=den, in1=m0[:],
                                        op=OP.add)
            rec = pool.tile([P, 1], f32, tag="rec")
            nc.vector.reciprocal(rec[:], dsafe[:])
            h_t = pool.tile([P, F], f32, tag="h")
            nc.vector.tensor_scalar(out=h_t[:], in0=acc[:, 0:F],
                                    scalar1=rec[:, 0:1], scalar2=None,
                                    op0=OP.mult)
            htp = psep.tile([F, P], f32, tag="htp")
            nc.tensor.transpose(out=htp[:], in_=h_t[:], identity=ident[:])
            ht = pool.tile([F, P], f32, tag="ht")
            nc.vector.tensor_copy(out=ht[:], in_=htp[:])
            op_t = psep.tile([P, F], f32, tag="op")
            nc.tensor.matmul(out=op_t[:], lhsT=ht[:], rhs=wmat_t[0:F, :],
                             start=True, stop=True)
            if g % 7 == 0:
                outs_sb = pool.tile([P, 7 * F], f32, tag="outs")
            if bzero:
                nc.vector.tensor_copy(
                    out=outs_sb[:, (g % 7) * F:(g % 7 + 1) * F],
                    in_=op_t[:])
            else:
                badd = pool.tile([P, F], f32, tag="badd")
                nc.vector.tensor_scalar(out=badd[:], in0=bv,
                                        scalar1=w1[:, 0:1], scalar2=None,
                                        op0=OP.mult)
                nc.vector.tensor_tensor(
                    out=outs_sb[:, (g % 7) * F:(g % 7 + 1) * F],
                    in0=op_t[:], in1=badd[:], op=OP.add)
            if g % 7 == 6:
                g0 = g - 6
                nc.sync.dma_start(
                    out=out3[g0:g0 + 7].rearrange("g p f -> p g f"),
                    in_=outs_sb[:].rearrange("p (g f) -> p g f", f=F))


# ----------------------------------------------------------------------------
# host-side preparation (index plumbing only)
# ----------------------------------------------------------------------------

def prep_inputs(feature_a, feature_b, W, b, a_vec, edges, node_num_a,
                ncores=NCORES):
    fa = np.asarray(feature_a, np.float32)
    fb = np.asarray(feature_b, np.float32)
    W = np.asarray(W, np.float32)
    b = np.asarray(b, np.float32)
    a_vec = np.asarray(a_vec, np.float32).reshape(-1)
    edges = np.asarray(edges)
    NA = int(node_num_a)
    NB, Fdim = fb.shape
    assert Fdim == F and fa.shape[1] == F

    a1 = a_vec[:F]
    a2 = a_vec[F:]
    Wa2 = (W @ a2).astype(np.float32)
    ba2 = float(b @ a2)

    nodes_per_core = -(-NA // (ncores * P)) * P          # 6272
    Gc = nodes_per_core // P                             # 49
    NA_pad = nodes_per_core * ncores

    H = -(-NB // 256) * 128                              # 25088 (<= 32768)
    assert H <= 32768 and NB <= 2 * H
    fb_pad = np.zeros((2 * H, F), np.float32)
    fb_pad[:NB] = fb

    src = edges[:, 0].astype(np.int64)
    dst = edges[:, 1].astype(np.int64)
    core = src // nodes_per_core
    half = (dst >= H).astype(np.int64)
    dloc = dst - half * H
    sl_node = src - core * nodes_per_core
    g_all = sl_node // P
    sloc_all = sl_node % P

    # per (core, half, group) counts -> cross-core uniform chunk counts
    cell = (core * 2 + half) * Gc + g_all
    counts = np.bincount(cell, minlength=ncores * 2 * Gc) \
        .reshape(ncores, 2, Gc)
    nch = np.maximum(1, -(-counts.max(axis=0) // P))     # [2, Gc]

    # chunk id layout per half: group-major chunk streams
    chunk_of_g = [np.concatenate([[0], np.cumsum(nch[X])]) for X in (0, 1)]
    nch_tot = [int(nch[X].sum()) for X in (0, 1)]
    NBATCH = [-(-nch_tot[X] // CB) for X in (0, 1)]

    # per-half metadata (identical for all cores)
    meta = []
    for X in (0, 1):
        chunk_groups = np.full(NBATCH[X] * CB, -1, np.int64)
        chunk_groups[:nch_tot[X]] = np.repeat(np.arange(Gc), nch[X])
        batches = []
        for bi in range(NBATCH[X]):
            cg = chunk_groups[bi * CB:(bi + 1) * CB]
            runs = []
            cprev = 0
            for c in range(1, CB + 1):
                if c == CB or cg[c] != cg[cprev]:
                    if cg[cprev] >= 0:
                        runs.append((cprev, c, int(cg[cprev])))
                    cprev = c
            batches.append(dict(runs=runs))
        group_chunks = []
        for g in range(Gc):
            lo, hi = chunk_of_g[X][g], chunk_of_g[X][g + 1]
            group_chunks.append([(int(ci // CB), int(ci % CB))
                                 for ci in range(lo, hi)])
        meta.append(dict(batches=batches, group_chunks=group_chunks))

    iota8 = np.tile(np.arange(P, dtype=np.float32)[None, :], (P, CB))

    wvec = np.zeros((P, 3 * F), np.float32)
    wvec[:, 0:F] = a1[None, :]
    wvec[:, F:2 * F] = Wa2[None, :]
    wvec[:, 2 * F:3 * F] = b[None, :]

    in_maps = []
    for c in range(ncores):
        msk = core == c
        ehalf = half[msk]
        edloc = dloc[msk]
        eg = g_all[msk]
        esloc = sloc_all[msk]
        order = np.lexsort((esloc, eg, ehalf))
        ehalf, edloc, eg, esloc = (x[order] for x in
                                   (ehalf, edloc, eg, esloc))
        gidx_maps = []
        gidx2_maps = []
        sloc_maps = []
        for X in (0, 1):
            nslot = NBATCH[X] * CB * P
            dsl = np.zeros(nslot, np.int64)
            nsl = np.zeros(nslot, np.int64)
            ssl = np.full(nslot, -1.0, np.float32)
            selX = ehalf == X
            dX = edloc[selX]
            gX = eg[selX]
            sX = esloc[selX]
            cnts = np.bincount(gX, minlength=Gc)
            off = np.concatenate([[0], np.cumsum(cnts)])
            for g in range(Gc):
                n = int(cnts[g])
                base = int(chunk_of_g[X][g]) * P
                dsl[base:base + n] = dX[off[g]:off[g] + n]
                nsl[base:base + n] = g * P + sX[off[g]:off[g] + n]
                ssl[base:base + n] = sX[off[g]:off[g] + n]
            # pack gather indices: flat[i = c*128+e]; 16-wrap + 8x tile
            dslb = dsl.reshape(NBATCH[X], CB * P)
            nslb = nsl.reshape(NBATCH[X], CB * P)
            gmaps = np.zeros((NBATCH[X], P, CB * P // 16), np.int16)
            g2maps = np.zeros((NBATCH[X], P, CB * P // 16), np.int16)
            smaps = np.zeros((NBATCH[X], P, CB), np.float32)
            for bi in range(NBATCH[X]):
                sb = dslb[bi].reshape(CB * P // 16, 16).T.astype(np.int16)
                gmaps[bi] = np.tile(sb, (8, 1))
                sb2 = nslb[bi].reshape(CB * P // 16, 16).T.astype(np.int16)
                g2maps[bi] = np.tile(sb2, (8, 1))
                smaps[bi] = ssl[bi * CB * P:(bi + 1) * CB * P] \
                    .reshape(CB, P).T
            gidx_maps.append(gmaps)
            gidx2_maps.append(g2maps)
            sloc_maps.append(smaps)

        fa_core = np.zeros((nodes_per_core, F), np.float32)
        lo = c * nodes_per_core
        hi = min(lo + nodes_per_core, NA)
        fa_core[:hi - lo] = fa[lo:hi]
        fa_pk = np.ascontiguousarray(
            fa_core.reshape(Gc, P, F).transpose(1, 0, 2).reshape(P, Gc * F))

        wmat_pad = np.zeros((P, F), np.float32)
        wmat_pad[0:F] = W
        consts = np.ascontiguousarray(
            np.concatenate([iota8, wvec, wmat_pad, fa_pk], axis=1))
        in_maps.append(dict(
            fb_pad=fb_pad,
            consts=consts,
            gidx_all=np.ascontiguousarray(
                np.concatenate([gidx_maps[0], gidx_maps[1]], axis=0)),
            sloc_all=np.ascontiguousarray(
                np.concatenate([sloc_maps[0], sloc_maps[1]], axis=0)),
            **(dict(gidx2A=gidx2_maps[0], gidx2B=gidx2_maps[1])
               if E1G else {}),
        ))

    cfg = dict(Gc=Gc, H=H, ba2=ba2, meta=meta, NBATCH=NBATCH,
               nodes_per_core=nodes_per_core, NA=NA,
               bzero=bool(np.all(b == 0.0)))
    return in_maps, cfg


def build_bass(cfg, ncores=NCORES):
    nc = bacc.Bacc("TRN2", target_bir_lowering=False, debug=False,
                   enable_asserts=False, num_devices=ncores,
                   num_swdge_queues=NQ)
    Gc, H = cfg["Gc"], cfg["H"]
    NBATCH = cfg["NBATCH"]
    NBT = NBATCH[0] + NBATCH[1]
    CW = CB * P + 3 * F + F + Gc * F
    ins = dict(
        fb_pad=nc.dram_tensor("fb_pad", [2 * H, F], f32,
                              kind="ExternalInput").ap(),
        consts=nc.dram_tensor("consts", [P, CW], f32,
                              kind="ExternalInput").ap(),
        gidx_all=nc.dram_tensor("gidx_all", [NBT, P, CB * P // 16], i16,
                                kind="ExternalInput").ap(),
        sloc_all=nc.dram_tensor("sloc_all", [NBT, P, CB], f32,
                                kind="ExternalInput").ap(),

        e1d=nc.dram_tensor("e1d", [Gc, P], f32, kind="Internal").ap(),

    )
    outs = dict(
        out=nc.dram_tensor("out", [Gc * P, F], f32,
                           kind="ExternalOutput").ap(),
    )
    with tile.TileContext(nc) as tc:
        emit_program(tc, ins, outs, cfg)
    nc.compile()
    return nc


# ----------------------------------------------------------------------------
# entry point
# ----------------------------------------------------------------------------

def kernel(**inputs):
    from concourse import bass_utils

    in_maps, cfg = prep_inputs(**inputs)
    nc = build_bass(cfg)
    res = bass_utils.run_bass_kernel_spmd(
        nc, in_maps, core_ids=list(range(NCORES)))
    outs = [r["out"][:cfg["nodes_per_core"]] for r in res.results]
    full = np.concatenate(outs, axis=0)[:cfg["NA"]]
    return full.astype(np.float32)


# ----------------------------------------------------------------------------
# numpy emulation of the device program (for host-prep validation)
# ----------------------------------------------------------------------------

def emulate(in_maps, cfg):
    Gc, H = cfg["Gc"], cfg["H"]
    meta = cfg["meta"]
    ba2 = cfg["ba2"]
    outs = []
    o_wvec = CB * P
    o_wmat, o_fa = CB * P + 3 * F, CB * P + 3 * F + F
    for im in in_maps:
        fb_pad = im["fb_pad"]
        consts = im["consts"]
        fa = consts[:, o_fa:o_fa + Gc * F].reshape(P, Gc, F) \
            .transpose(1, 0, 2).reshape(Gc * P, F)
        a1 = consts[0, o_wvec:o_wvec + F]
        Wa2 = consts[0, o_wvec + F:o_wvec + 2 * F]
        bvec = consts[0, o_wvec + 2 * F:o_wvec + 3 * F]
        W = consts[0:F, o_wmat:o_wmat + F]
        e1 = fa @ a1 + ba2                       # [Gc*P]
        acc = np.zeros((Gc * P, F + 1), np.float64)
        nb0 = cfg["NBATCH"][0]
        for X, ofs in ((0, 0), (1, nb0)):
            NBx = cfg["NBATCH"][X]
            gm = im["gidx_all"][ofs:ofs + NBx]
            sm = im["sloc_all"][ofs:ofs + NBx]
            for bi in range(NBx):
                # unpack idx: [128, CB*P//16] -> flat
                sb = gm[bi][:16]                 # [16, CB*P//16]
                flat = sb.T.reshape(-1).astype(np.int64)   # i = c*128+e
                didx = flat.reshape(CB, P)       # [c, e]
                sl = sm[bi]                      # [e, c]
                rows = fb_pad[X * H + didx]      # [c, e, F]
                q = rows @ Wa2                   # [c, e]
                for (c0, c1, g) in meta[X]["batches"][bi]["runs"]:
                    for c in range(c0, c1):
                        sle = sl[:, c]
                        valid = sle >= 0
                        s = q[c] + e1[g * P + (sle.astype(np.int64) % P)]
                        score = np.where(
                            s > 0, np.exp(s),
                            np.exp(0.1 * np.exp(np.minimum(s, 0)) - 0.1))
                        score = score * valid
                        onehot = (sle[:, None] ==
                                  np.arange(P)[None, :])   # [e, n]
                        acc[g * P:(g + 1) * P, 0:F] += \
                            onehot.T @ (score[:, None] * rows[c])
                        acc[g * P:(g + 1) * P, F] += onehot.T @ score
        den = acc[:, F]
        dsafe = np.where(den == 0, 1.0, den)
        h = acc[:, 0:F] / dsafe[:, None]
        o = h @ W + (den > 0)[:, None] * bvec[None, :]
        outs.append(o.astype(np.float32))
    full = np.concatenate(outs, 0)[:cfg["NA"]]
    return full


if __name__ == "__main__":
    np.random.seed(0)
    NA = NB = 50000
    E = 800000
    ins = dict(
        feature_a=np.random.randn(NA, F).astype(np.float32),
        feature_b=np.random.randn(NB, F).astype(np.float32),
        W=(np.random.randn(F, F) / 8).astype(np.float32),
        b=(np.random.randn(F) * 0.1).astype(np.float32),
        a_vec=(np.random.randn(2 * F, 1) * 0.05).astype(np.float32),
        edges=np.stack([np.random.randint(0, NA, E),
                        np.random.randint(0, NB, E)], 1).astype(np.int64),
        node_num_a=NA,
    )
    # numpy reference
    fa, fb = ins["feature_a"], ins["feature_b"]
    W, b_, av = ins["W"], ins["b"], ins["a_vec"].reshape(-1)
    src, dst = ins["edges"][:, 0], ins["edges"][:, 1]
    new_emb = fb @ W + b_
    s = (fa @ av[:F])[src] + (new_emb @ av[F:])[dst]
    score = np.exp(np.where(s > 0, s, 0.1 * (np.exp(np.minimum(s, 0)) - 1)))
    num = np.zeros((NA, F))
    np.add.at(num, src, score[:, None] * new_emb[dst])
    den = np.zeros(NA)
    np.add.at(den, src, score)
    dsafe = np.where(den == 0, 1, den)
    expected = num / dsafe[:, None]

    in_maps, cfg = prep_inputs(**ins)
    print("NBATCH:", cfg["NBATCH"], "slots:",
          sum(n * CB * P for n in cfg["NBATCH"]))
    got = emulate(in_maps, cfg)
    rel = np.linalg.norm(got - expected) / np.linalg.norm(expected)
    print("emulation rel err:", rel)
